# revision 5
# baseline (speedup 1.0000x reference)
"""Trainium2 Bass kernel for nn_ConvNet (char-CNN word encoder + sentence conv + MLP).

Single-stream char path: one one-hot stream X[128 vocab, SX] fp8 with one
zero pad column between words (33 cols/word).  The 3-tap conv collapses to
shifted one-hot matmuls into the same PSUM bank:
    out[j] = ET0.X[j] + ET1'.X[j+1] + ET2.X[j+2]
where ET1' has the conv bias folded in (center tap is never a pad col for a
real output).  The DoubleRow fp8 pass computes (ET0, ET1') via an
OVERLAPPING rhs AP (slots read X[j], X[j+1] from the same stream); the
third tap is a normal fp8 matmul on X[j+2:].  Word isolation is automatic:
pad cols contribute exactly 0, and outputs at pad positions (j%33==32) are
garbage that the strided max simply skips.

PSUM drain (the DVE bottleneck in v1) is split: per 5-bank supergroup, one
bank is max-reduced directly from PSUM by the DVE; four banks are evacuated
by the Scalar engine (strided compact copy to bf16 SBUF) and reduced by a
DVE tensor_tensor max tree, which runs in the 2x_1p DVE mode (2 elem/cyc)
unlike tensor_reduce (1x only).

The sentence conv (bf16, 48 matmuls) is interleaved into the char phase in
4 chunks of 128 words so the PE never idles (p-state stays at 2.4 GHz) and
only the last chunk is tail-exposed.  The 8-way max uses two AllGathers of
[128,4] partial maxes: phase A (chunks 0-2) launches ~85% through the char
phase and hides its latency; phase B (chunk 3) pays only its own sync.
"""

import sys

try:
    import concourse  # noqa: F401
except ImportError:
    sys.path.insert(0, "/opt/trn_rl_repo")

import numpy as np
import ml_dtypes

import concourse.bass as bass
import concourse.bacc as bacc
import concourse.tile as tile
from concourse import mybir
from concourse.bass_utils import run_bass_kernel_spmd
from bass_rust import VecI64Pair

BF16 = ml_dtypes.bfloat16
FP8 = ml_dtypes.float8_e4m3

CORES = 8
D = 256
L = 32
PW = L + 1          # stream cols per word (32 chars + 1 pad)
WB = 15             # words per PSUM bank (15*33 = 495 <= 512)
ETS = 64.0          # fp8 scale for the ET response tables


def _shapes(W):
    WPC = W // CORES            # real words per core
    NW = WPC + 2                # + 1 halo word each side
    SX = PW * NW + 1 + 2        # stream cols (+lead pad, +2 tail zeros)
    SXP = -(-SX // 16) * 16     # padded to 16
    NB = -(-NW // WB)           # banks per m-half
    G = -(-NW // 128)           # word-gather groups of 128
    return WPC, NW, SX, SXP, NB, G


def build(W):
    WPC, NW, SX, SXP, NB, G = _shapes(W)
    f32 = mybir.dt.float32
    bf16 = mybir.dt.bfloat16
    f8 = mybir.dt.float8e4
    i32 = mybir.dt.int32

    # banks: (word0, nwords); supergroups of 5 banks (last ragged)
    banks = []
    w0 = 0
    while w0 < NW:
        banks.append((w0, min(WB, NW - w0)))
        w0 += WB
    sgs = [banks[i : i + 5] for i in range(0, len(banks), 5)]

    # sentence chunks of 128 real words; chunk c ready after u words
    # [0, 128c+130) exist
    CH = 128
    NCH = WPC // CH
    UPAD = -(-NW // 16) * 16

    nc = bacc.Bacc(num_devices=CORES)

    onehot = nc.declare_dram_parameter("onehot", [128, SXP], f8, isOutput=False)
    wdr = nc.declare_dram_parameter("wdr", [128, 2, 256], f8, isOutput=False)
    wn = nc.declare_dram_parameter("wn", [128, 2, 128], f8, isOutput=False)
    widx = nc.declare_dram_parameter("widx", [128, G], i32, isOutput=False)
    wemb = nc.declare_dram_parameter("wemb", [50000, D], f32, isOutput=False)
    wsT = nc.declare_dram_parameter("wsT", [128, 3, 4, 2 * D], bf16, isOutput=False)
    bsent = nc.declare_dram_parameter("bsent", [128, 4], f32, isOutput=False)
    w1t = nc.declare_dram_parameter("w1t", [128, 4, 8, 128], bf16, isOutput=False)
    b1t = nc.declare_dram_parameter("b1t", [128, 8], f32, isOutput=False)
    w2t = nc.declare_dram_parameter("w2t", [128, 8, 2], bf16, isOutput=False)
    b2t = nc.declare_dram_parameter("b2t", [2, 1], f32, isOutput=False)
    ident = nc.declare_dram_parameter("ident", [128, 128], f32, isOutput=False)
    out = nc.declare_dram_parameter("out", [2, 1], f32, isOutput=True)

    def dr_rhs(x_sb, a0, n):
        # overlapping DoubleRow rhs: slots read X[:, a0+j] and X[:, a0+j+1]
        v = x_sb[:, a0 : a0 + n].unsqueeze(1).broadcast_to([128, 2, n])
        v.ap = VecI64Pair([[SXP, 128], [1, 2], [1, n]])
        return v

    with tile.TileContext(nc) as tc:
        with (
            tc.tile_pool(name="const", bufs=1) as cpool,
            tc.tile_pool(name="gath", bufs=G) as gpool,
            tc.tile_pool(name="strip", bufs=4) as spool,
            tc.tile_pool(name="tree", bufs=2) as tpool,
            tc.tile_pool(name="dram", bufs=1, space="DRAM") as dpool,
        ):
            x_sb = cpool.tile([128, SXP], f8, name="x_sb")
            wdr_sb = cpool.tile([128, 2, 256], f8)
            wn_sb = cpool.tile([128, 2, 128], f8)
            widx_sb = cpool.tile([128, G], i32)
            wsT_sb = cpool.tile([128, 3, 4, 2 * D], bf16)
            ident_sb = cpool.tile([128, 128], f32)
            bsent_sb = cpool.tile([128, 4], f32)
            w1t_sb = cpool.tile([128, 4, 8, 128], bf16)
            b1t_sb = cpool.tile([128, 8], f32)
            w2t_sb = cpool.tile([128, 8, 2], bf16)
            b2t_sb = cpool.tile([2, 1], f32)
            u = cpool.tile([128, 4, UPAD], bf16, name="u")

            dum_in = dpool.tile([128, 2], f32)
            dum_out = nc.dram_tensor("dum_out", [CORES, 128, 2], f32, addr_space="Shared")

            # ---- head DMAs -------------------------------------------------
            # weights first (tiny) so bank-0 matmuls can start immediately
            nc.scalar.dma_start(out=wdr_sb[:], in_=wdr[:])
            nc.scalar.dma_start(out=wn_sb[:], in_=wn[:])
            # big sentence weights: one async transfer issued early
            nc.scalar.dma_start(out=wsT_sb[:], in_=wsT[:])
            # one-hot stream sections, alternating sync/gpsimd; first small
            secs = [0, 2 * 495 + 2]
            s = secs[-1]
            step = -(-(SXP - s) // 6)
            while s < SXP:
                s = min(s + step, SXP)
                secs.append(s)
            for si, (a, b) in enumerate(zip(secs[:-1], secs[1:])):
                eng = nc.sync if si % 2 == 0 else nc.gpsimd
                eng.dma_start(out=x_sb[:, a:b], in_=onehot[:, a:b])
            nc.gpsimd.dma_start(out=widx_sb[:], in_=widx[:])
            # word-embedding gathers (independent of char path)
            wrd_sb = []
            for g in range(G):
                wt = gpool.tile([128, D], f32, tag="wrd")
                nc.gpsimd.indirect_dma_start(
                    out=wt[:],
                    out_offset=None,
                    in_=wemb[:],
                    in_offset=bass.IndirectOffsetOnAxis(ap=widx_sb[:, g : g + 1], axis=0),
                )
                wrd_sb.append(wt)
            nc.sync.dma_start(out=ident_sb[:], in_=ident[:])
            # warm-up collective: arms ncfw so the real AllGathers are cheap
            nc.gpsimd.dma_start(out=dum_in[:], in_=widx_sb[:].bitcast(f32)[:, 0:2])
            nc.gpsimd.collective_compute(
                "AllGather",
                mybir.AluOpType.bypass,
                replica_groups=[list(range(CORES))],
                ins=[dum_in[:]],
                outs=[dum_out[:]],
            )
            nc.gpsimd.dma_start(out=bsent_sb[:], in_=bsent[:])
            nc.gpsimd.dma_start(out=b1t_sb[:], in_=b1t[:])
            nc.gpsimd.dma_start(out=b2t_sb[:], in_=b2t[:])
            nc.gpsimd.dma_start(out=w1t_sb[:], in_=w1t[:])
            nc.gpsimd.dma_start(out=w2t_sb[:], in_=w2t[:])

            # ---- char pipeline + interleaved sentence conv -----------------
            rch = cpool.tile([128, 4, NCH], f32, name="rch")
            ccA_in = dpool.tile([128, 4], f32)
            ccB_in = dpool.tile([128, 4], f32)
            ccA_out = nc.dram_tensor("ccA_out", [CORES, 128, 4], f32, addr_space="Shared")
            ccB_out = nc.dram_tensor("ccB_out", [CORES, 128, 4], f32, addr_space="Shared")

            def char_bank(pch, m, w0, nw):
                n = PW * nw
                a0 = PW * w0
                py = pch.tile([128, 512], f32, tag="py")
                nc.tensor.matmul(
                    out=py[:, :n],
                    lhsT=wdr_sb[:, m, :],
                    rhs=dr_rhs(x_sb, a0, n),
                    start=True,
                    stop=False,
                    perf_mode=mybir.MatmulPerfMode.DoubleRowSwInterleave,
                )
                nc.tensor.matmul(
                    out=py[:, :n],
                    lhsT=wn_sb[:, m, :],
                    rhs=x_sb[:, a0 + 2 : a0 + 2 + n],
                    start=False,
                    stop=True,
                )
                return py

            def drain_direct(py, m, w0, nw):
                pv = py[:, : PW * nw].rearrange("p (w t) -> p w t", t=PW)
                nc.vector.tensor_reduce(
                    out=u[:, 2 + m, w0 : w0 + nw],
                    in_=pv[:, :, 0:L],
                    axis=mybir.AxisListType.X,
                    op=mybir.AluOpType.max,
                )

            def drain_evac(py, strip, j, nw):
                pv = py[:, : PW * nw].rearrange("p (w t) -> p w t", t=PW)
                nc.scalar.activation(
                    out=strip[:, j, :nw, :],
                    in_=pv[:, :, 0:L],
                    func=mybir.ActivationFunctionType.Copy,
                )

            def tree(strip, m, w0, nwords):
                # strip [128, 4, 15, 32] holding nwords valid words
                sv = strip[:].rearrange("p b w t -> p (b w) t")[:, :nwords, :]
                l1 = tpool.tile([128, 60, 16], bf16, tag="l1", name="l1")
                l2 = tpool.tile([128, 60, 8], bf16, tag="l2", name="l2")
                l3 = tpool.tile([128, 60, 4], bf16, tag="l3", name="l3")
                l4 = tpool.tile([128, 60, 2], bf16, tag="l4", name="l4")
                nc.vector.tensor_tensor(
                    out=l1[:, :nwords, :], in0=sv[:, :, 0:16], in1=sv[:, :, 16:32],
                    op=mybir.AluOpType.max)
                nc.vector.tensor_tensor(
                    out=l2[:, :nwords, :], in0=l1[:, :nwords, 0:8],
                    in1=l1[:, :nwords, 8:16], op=mybir.AluOpType.max)
                nc.vector.tensor_tensor(
                    out=l3[:, :nwords, :], in0=l2[:, :nwords, 0:4],
                    in1=l2[:, :nwords, 4:8], op=mybir.AluOpType.max)
                nc.vector.tensor_tensor(
                    out=l4[:, :nwords, :], in0=l3[:, :nwords, 0:2],
                    in1=l3[:, :nwords, 2:4], op=mybir.AluOpType.max)
                nc.vector.tensor_tensor(
                    out=u[:, 2 + m, w0 : w0 + nwords], in0=l4[:, :nwords, 0],
                    in1=l4[:, :nwords, 1], op=mybir.AluOpType.max)

            def transposes():
                with tc.tile_pool(name="ptp", bufs=2, space="PSUM") as ptp:
                    for g in range(G):
                        wcnt = min(128, NW - g * 128)
                        for cc in range(2):
                            tp = ptp.tile([128, 128], f32, tag="tp")
                            nc.tensor.transpose(
                                out=tp[:],
                                in_=wrd_sb[g][:, cc * 128 : (cc + 1) * 128],
                                identity=ident_sb[:],
                            )
                            nc.scalar.activation(
                                out=u[:, cc, g * 128 : g * 128 + wcnt],
                                in_=tp[:, :wcnt],
                                func=mybir.ActivationFunctionType.Copy,
                            )

            def sent_chunk(psn, c):
                # r[:, w] for w in [128c, 128c+128): u cols [w, w+3)
                ps = psn.tile([128, 4, CH], f32, tag="ps")
                for mb in range(4):
                    first = True
                    for k in range(3):
                        for kc in range(4):
                            nc.tensor.matmul(
                                out=ps[:, mb, :],
                                lhsT=wsT_sb[:, k, kc, mb * 128 : (mb + 1) * 128],
                                rhs=u[:, kc, c * CH + k : c * CH + k + CH],
                                start=first,
                                stop=(k == 2 and kc == 3),
                            )
                            first = False
                for mb in range(4):
                    nc.vector.tensor_reduce(
                        out=rch[:, mb, c : c + 1],
                        in_=ps[:, mb, :],
                        axis=mybir.AxisListType.X,
                        op=mybir.AluOpType.max,
                    )

            # chunk c fires once u words [0, 128c+130) are done; with 75
            # words per supergroup that's after sg (128c+130)/75
            chunk_after_sg = {}
            for c in range(NCH - 1):
                chunk_after_sg.setdefault(min(-(-(CH * c + CH + 2) // 75), len(sgs)) - 1, []).append(c)

            with nc.named_scope("char"):
                with (
                    tc.tile_pool(name="pch", bufs=4, space="PSUM") as pch,
                    tc.tile_pool(name="psn", bufs=1, space="PSUM") as psn,
                ):
                    for sgi, sg in enumerate(sgs):
                        strips = []
                        for m in range(2):
                            if len(sg) > 1:
                                strips.append(spool.tile([128, 4, WB, L], bf16, tag=f"s{m}", name=f"strip{m}"))
                            else:
                                strips.append(None)
                        for bi, (w0, nw) in enumerate(sg):
                            for m in range(2):
                                py = char_bank(pch, m, w0, nw)
                                if bi == 0:
                                    drain_direct(py, m, w0, nw)
                                else:
                                    drain_evac(py, strips[m], bi - 1, nw)
                        sg_w0 = sg[1][0] if len(sg) > 1 else None
                        sg_nw = sum(nw for (_, nw) in sg[1:])
                        for m in range(2):
                            if strips[m] is not None:
                                tree(strips[m], m, sg_w0, sg_nw)
                        if sgi == 0:
                            transposes()
                        for c in chunk_after_sg.get(sgi, []):
                            sent_chunk(psn, c)
                            if c == NCH - 2:
                                # phase A: max over chunks 0..NCH-2 -> gather
                                ra = cpool.tile([128, 4], f32)
                                nc.vector.tensor_reduce(
                                    out=ra[:],
                                    in_=rch[:, :, : NCH - 1],
                                    axis=mybir.AxisListType.X,
                                    op=mybir.AluOpType.max,
                                )
                                nc.gpsimd.dma_start(out=ccA_in[:], in_=ra[:])
                                nc.gpsimd.collective_compute(
                                    "AllGather",
                                    mybir.AluOpType.bypass,
                                    replica_groups=[list(range(CORES))],
                                    ins=[ccA_in[:]],
                                    outs=[ccA_out[:]],
                                )
                    # last sentence chunk + phase B
                    sent_chunk(psn, NCH - 1)
                    nc.gpsimd.dma_start(
                        out=ccB_in[:], in_=rch[:, :, NCH - 1 : NCH].rearrange("p m c -> p (m c)")
                    )
                    nc.gpsimd.collective_compute(
                        "AllGather",
                        mybir.AluOpType.bypass,
                        replica_groups=[list(range(CORES))],
                        ins=[ccB_in[:]],
                        outs=[ccB_out[:]],
                    )

            # ---- global max + MLP -----------------------------------------
            rg = cpool.tile([128, 2, CORES, 4], f32)
            nc.sync.dma_start(out=rg[:, 0], in_=ccA_out[:].rearrange("r p f -> p r f"))
            nc.sync.dma_start(out=rg[:, 1], in_=ccB_out[:].rearrange("r p f -> p r f"))
            rmax = cpool.tile([128, 4], f32)
            nc.vector.tensor_reduce(
                out=rmax[:],
                in_=rg[:].rearrange("p g r f -> p f g r"),
                axis=mybir.AxisListType.XY,
                op=mybir.AluOpType.max,
            )
            r_sb = cpool.tile([128, 4], bf16)
            nc.vector.tensor_tensor(
                out=r_sb[:], in0=rmax[:], in1=bsent_sb[:], op=mybir.AluOpType.add
            )
            with tc.tile_pool(name="pmlp", bufs=2, space="PSUM") as pmlp:
                hp = pmlp.tile([128, 8], f32, tag="hp")
                for j in range(8):
                    for k in range(4):
                        nc.tensor.matmul(
                            out=hp[:, j : j + 1],
                            lhsT=w1t_sb[:, k, j, :],
                            rhs=r_sb[:, k : k + 1],
                            start=(k == 0),
                            stop=(k == 3),
                        )
                hb = cpool.tile([128, 8], f32)
                nc.vector.tensor_tensor(
                    out=hb[:], in0=hp[:], in1=b1t_sb[:], op=mybir.AluOpType.add
                )
                h_sb = cpool.tile([128, 8], bf16)
                nc.scalar.activation(
                    out=h_sb[:], in_=hb[:], func=mybir.ActivationFunctionType.Tanh
                )
                o_ps = pmlp.tile([2, 1], f32, tag="op")
                for k in range(8):
                    nc.tensor.matmul(
                        out=o_ps[:],
                        lhsT=w2t_sb[:, k, :],
                        rhs=h_sb[:, k : k + 1],
                        start=(k == 0),
                        stop=(k == 7),
                    )
                o_sb = cpool.tile([2, 1], f32)
                nc.vector.tensor_tensor(
                    out=o_sb[:], in0=o_ps[:], in1=b2t_sb[:], op=mybir.AluOpType.add
                )
                nc.sync.dma_start(out=out[:], in_=o_sb[:])

    nc.finalize()
    return nc


def prep_in_maps(words, words_in_char, word_emb, chr_emb, conv_chr_w, conv_chr_b,
                 conv_sent_w, conv_sent_b, w1, b1, w2, b2):
    W = words.shape[0]
    WPC, NW, SX, SXP, NB, G = _shapes(W)

    words = np.asarray(words, np.int32)
    chars = np.asarray(words_in_char, np.int32)
    word_emb = np.asarray(word_emb, np.float32)
    chr_emb = np.asarray(chr_emb, np.float32)
    conv_chr_w = np.asarray(conv_chr_w, np.float32)
    conv_chr_b = np.asarray(conv_chr_b, np.float32)
    conv_sent_w = np.asarray(conv_sent_w, np.float32)
    conv_sent_b = np.asarray(conv_sent_b, np.float32)
    w1 = np.asarray(w1, np.float32)
    b1 = np.asarray(b1, np.float32)
    w2 = np.asarray(w2, np.float32)
    b2 = np.asarray(b2, np.float32)

    # host-folded char response tables (x64 into fp8 range); conv bias is
    # folded into the center tap's table (center col is never a pad col)
    ET = [ETS * (chr_emb @ conv_chr_w[:, :, k].T) for k in range(3)]
    ET[1] = ET[1] + ETS * conv_chr_b[None, :]
    ET = [t.astype(FP8).astype(np.float32) for t in ET]

    # DR SWI weights for (ET0, ET1'), per m-half; normal weights = ET2
    wdr = np.zeros((128, 2, 256), np.float32)
    wn = np.zeros((128, 2, 128), np.float32)
    for m in range(2):
        a = ET[0][:, m * 128 : (m + 1) * 128]
        b = ET[1][:, m * 128 : (m + 1) * 128]
        wdr[:, m, :] = np.stack([a[:, ::-1], b[:, ::-1]], axis=2).reshape(128, 256)
        wn[:, m, :] = ET[2][:, m * 128 : (m + 1) * 128]

    # sentence conv: char-half input channels absorb the 1/64
    ws = conv_sent_w.copy()
    ws[:, D:, :] /= ETS
    wsT = np.ascontiguousarray(
        ws.transpose(1, 2, 0).reshape(4, 128, 3, 2 * D).transpose(1, 2, 0, 3)
    ).astype(BF16)                                        # [p, k, kc, c2]
    bsent = np.ascontiguousarray(conv_sent_b.reshape(4, 128).T).astype(np.float32)
    w1t = np.ascontiguousarray(
        w1.reshape(8, 128, 4, 128).transpose(3, 2, 0, 1)
    ).astype(BF16)                                  # [p, k, m, c]
    b1t = np.ascontiguousarray(b1.reshape(8, 128).T).astype(np.float32)
    w2t = np.ascontiguousarray(
        w2.T.reshape(8, 128, 2).transpose(1, 0, 2)
    ).astype(BF16)                                  # [p, k, j]
    b2t = b2.reshape(2, 1).astype(np.float32)
    ident = np.eye(128, dtype=np.float32)

    in_maps = []
    for c in range(CORES):
        lo = c * WPC - 1
        idxs = np.arange(lo, lo + NW)
        valid = (idxs >= 0) & (idxs < W)
        w_ext = np.where(valid, words[np.clip(idxs, 0, W - 1)], 0).astype(np.int32)
        ch_ext = np.zeros((NW, L), np.int32)
        ch_ext[valid] = chars[np.clip(idxs, 0, W - 1)[valid]]

        # one-hot stream: word w chars at cols PW*w+1 .. PW*w+32; halo words
        # outside [0, W) stay all-zero (their u must be exactly 0)
        oh = np.zeros((128, SXP), FP8)
        cols = (PW * np.arange(NW)[:, None] + 1 + np.arange(L)[None, :])
        vmask = np.repeat(valid, L)
        cf = cols.reshape(-1)[vmask]
        chf = ch_ext.reshape(-1)[vmask]
        oh[chf, cf] = 1.0

        wpad = np.zeros(G * 128, np.int32)
        wpad[:NW] = w_ext
        widx = np.ascontiguousarray(wpad.reshape(G, 128).T)

        in_maps.append(
            dict(
                onehot=oh,
                wdr=wdr.astype(FP8),
                wn=wn.astype(FP8),
                widx=widx,
                wemb=word_emb,
                wsT=wsT,
                bsent=bsent,
                w1t=w1t,
                b1t=b1t,
                w2t=w2t,
                b2t=b2t,
                ident=ident,
            )
        )
    return in_maps


_CACHE = {}


def _get_nc(W):
    if W not in _CACHE:
        _CACHE[W] = build(W)
    return _CACHE[W]


def run(inputs, trace=False):
    W = np.asarray(inputs["words"]).shape[0]
    nc = _get_nc(W)
    in_maps = prep_in_maps(**inputs)
    res = run_bass_kernel_spmd(nc, in_maps, list(range(CORES)), trace=trace)
    out = np.asarray(res.results[0]["out"], np.float32).reshape(1, 2)
    return out, res


def kernel(**inputs) -> np.ndarray:
    out, _ = run(inputs, trace=False)
    return out


# revision 12
# speedup vs baseline: 1.1351x; 1.1351x over previous
"""Trainium2 Bass kernel for nn_ConvNet (char-CNN word encoder + sentence conv + MLP).

Single-stream char path: one one-hot stream X[128 vocab, SX] fp8 with one
zero pad column between words (33 cols/word).  The 3-tap conv collapses to
shifted one-hot matmuls into the same PSUM bank:
    out[j] = ET0.X[j] + ET1'.X[j+1] + ET2.X[j+2]
where ET1' has the conv bias folded in (center tap is never a pad col for a
real output).  The DoubleRow fp8 pass computes (ET0, ET1') via an
OVERLAPPING rhs AP (slots read X[j], X[j+1] from the same stream); the
third tap is a normal fp8 matmul on X[j+2:].  Word isolation is automatic:
pad cols contribute exactly 0, and outputs at pad positions (j%33==32) are
garbage that the strided max simply skips.

PSUM drain (the DVE bottleneck in v1) is split: per 5-bank supergroup, one
bank is max-reduced directly from PSUM by the DVE; four banks are evacuated
by the Scalar engine (strided compact copy to bf16 SBUF) and reduced by a
DVE tensor_tensor max tree, which runs in the 2x_1p DVE mode (2 elem/cyc)
unlike tensor_reduce (1x only).

The sentence conv (bf16, 48 matmuls) is interleaved into the char phase in
4 chunks of 128 words so the PE never idles (p-state stays at 2.4 GHz) and
only the last chunk is tail-exposed.  The 8-way max uses two AllGathers of
[128,4] partial maxes: phase A (chunks 0-2) launches ~85% through the char
phase and hides its latency; phase B (chunk 3) pays only its own sync.
"""

import sys

try:
    import concourse  # noqa: F401
except ImportError:
    sys.path.insert(0, "/opt/trn_rl_repo")

import numpy as np
import ml_dtypes

import concourse.bass as bass
import concourse.bacc as bacc
import concourse.tile as tile
from concourse import mybir
from concourse.bass_utils import run_bass_kernel_spmd
from bass_rust import VecI64Pair

BF16 = ml_dtypes.bfloat16
FP8 = ml_dtypes.float8_e4m3

CORES = 8
D = 256
L = 32
PW = L + 1          # stream cols per word (32 chars + 1 pad)
WB = 15             # words per PSUM bank (15*33 = 495 <= 512)
ETS = 64.0          # fp8 scale for the ET response tables


def _shapes(W):
    WPC = W // CORES            # real words per core
    NW = WPC + 2                # + 1 halo word each side
    SX = PW * NW + 1 + 2        # stream cols (+lead pad, +2 tail zeros)
    SXP = -(-SX // 16) * 16     # padded to 16
    NB = -(-NW // WB)           # banks per m-half
    G = -(-NW // 128)           # word-gather groups of 128
    return WPC, NW, SX, SXP, NB, G


def build(W):
    WPC, NW, SX, SXP, NB, G = _shapes(W)
    f32 = mybir.dt.float32
    bf16 = mybir.dt.bfloat16
    f8 = mybir.dt.float8e4
    i32 = mybir.dt.int32

    # banks: (word0, nwords); supergroups of 5 banks (last ragged)
    banks = []
    w0 = 0
    while w0 < NW:
        banks.append((w0, min(WB, NW - w0)))
        w0 += WB
    sgs = [banks[i : i + 5] for i in range(0, len(banks), 5)]

    # sentence chunks of ~WPC/3 real words; chunk c ready after u words
    # [0, CH(c+1)+2) exist
    CH = 128
    NCH = -(-WPC // CH)
    UPAD = -(-NW // 16) * 16

    nc = bacc.Bacc(num_devices=CORES)

    onehot = nc.declare_dram_parameter("onehot", [128, SXP], f8, isOutput=False)
    wdr = nc.declare_dram_parameter("wdr", [128, 2, 256], f8, isOutput=False)
    wn = nc.declare_dram_parameter("wn", [128, 2, 128], f8, isOutput=False)
    widx = nc.declare_dram_parameter("widx", [128, G], i32, isOutput=False)
    wemb = nc.declare_dram_parameter("wemb", [50000, D], f32, isOutput=False)
    wsT = nc.declare_dram_parameter("wsT", [128, 3, 4, 2 * D], bf16, isOutput=False)
    bsent = nc.declare_dram_parameter("bsent", [128, 4], f32, isOutput=False)
    w1t = nc.declare_dram_parameter("w1t", [128, 4, 8, 128], bf16, isOutput=False)
    b1t = nc.declare_dram_parameter("b1t", [128, 8], f32, isOutput=False)
    w2t = nc.declare_dram_parameter("w2t", [128, 8, 2], bf16, isOutput=False)
    b2t = nc.declare_dram_parameter("b2t", [2, 1], f32, isOutput=False)
    ident = nc.declare_dram_parameter("ident", [128, 128], f32, isOutput=False)
    out = nc.declare_dram_parameter("out", [2, 1], f32, isOutput=True)

    def dr_rhs(x_sb, a0, n):
        # overlapping DoubleRow rhs: slots read X[:, a0+j] and X[:, a0+j+1]
        v = x_sb[:, a0 : a0 + n].unsqueeze(1).broadcast_to([128, 2, n])
        v.ap = VecI64Pair([[SXP, 128], [1, 2], [1, n]])
        return v

    with tile.TileContext(nc) as tc:
        with (
            tc.tile_pool(name="const", bufs=1) as cpool,
            tc.tile_pool(name="gath", bufs=G) as gpool,
            tc.tile_pool(name="strip", bufs=4) as spool,
            tc.tile_pool(name="tree", bufs=2) as tpool,
            tc.tile_pool(name="dram", bufs=1, space="DRAM") as dpool,
        ):
            x_sb = cpool.tile([128, SXP], f8, name="x_sb")
            wdr_sb = cpool.tile([128, 2, 256], f8)
            wn_sb = cpool.tile([128, 2, 128], f8)
            widx_sb = cpool.tile([128, G], i32)
            wsT_sb = cpool.tile([128, 3, 4, 2 * D], bf16)
            ident_sb = cpool.tile([128, 128], f32)
            bsent_sb = cpool.tile([128, 4], f32)
            w1t_sb = cpool.tile([128, 4, 8, 128], bf16)
            b1t_sb = cpool.tile([128, 8], f32)
            w2t_sb = cpool.tile([128, 8, 2], bf16)
            b2t_sb = cpool.tile([2, 1], f32)
            u = cpool.tile([128, 4, UPAD], bf16, name="u")

            dum_in = dpool.tile([128, 2], f32)
            dum_out = nc.dram_tensor("dum_out", [CORES, 128, 2], f32, addr_space="Shared")

            # ---- head DMAs -------------------------------------------------
            # weights first (tiny) so bank-0 matmuls can start immediately
            nc.scalar.dma_start(out=wdr_sb[:], in_=wdr[:])
            nc.scalar.dma_start(out=wn_sb[:], in_=wn[:])
            # big sentence weights: one async transfer issued early
            nc.scalar.dma_start(out=wsT_sb[:], in_=wsT[:])
            # widx first so the word-emb gathers can start early
            nc.gpsimd.dma_start(out=widx_sb[:], in_=widx[:])
            # one-hot stream sections, alternating sync/gpsimd; first small
            secs = [0, 2 * 495 + 2]
            s = secs[-1]
            step = -(-(SXP - s) // 6)
            while s < SXP:
                s = min(s + step, SXP)
                secs.append(s)
            wrd_sb = [gpool.tile([128, D], f32, tag="wrd", name=f"wrd{g}") for g in range(G)]

            def gather(g):
                nc.gpsimd.indirect_dma_start(
                    out=wrd_sb[g][:],
                    out_offset=None,
                    in_=wemb[:],
                    in_offset=bass.IndirectOffsetOnAxis(ap=widx_sb[:, g : g + 1], axis=0),
                )

            gather(0)
            gather(1)
            for si, (a, b) in enumerate(zip(secs[:-1], secs[1:])):
                eng = nc.sync if si % 2 == 0 else nc.gpsimd
                eng.dma_start(out=x_sb[:, a:b], in_=onehot[:, a:b])
                if si == 1:
                    gather(2)
                if si == 3:
                    gather(3)
                    gather(4)
            nc.sync.dma_start(out=ident_sb[:], in_=ident[:])
            # warm-up collective: arms ncfw so the real AllGathers are cheap
            nc.gpsimd.dma_start(out=dum_in[:], in_=widx_sb[:].bitcast(f32)[:, 0:2])
            nc.gpsimd.collective_compute(
                "AllGather",
                mybir.AluOpType.bypass,
                replica_groups=[list(range(CORES))],
                ins=[dum_in[:]],
                outs=[dum_out[:]],
            )
            nc.gpsimd.dma_start(out=bsent_sb[:], in_=bsent[:])
            nc.gpsimd.dma_start(out=b1t_sb[:], in_=b1t[:])
            nc.gpsimd.dma_start(out=b2t_sb[:], in_=b2t[:])
            nc.gpsimd.dma_start(out=w1t_sb[:], in_=w1t[:])
            nc.gpsimd.dma_start(out=w2t_sb[:], in_=w2t[:])

            # ---- char pipeline + interleaved sentence conv -----------------
            rch = cpool.tile([128, 4, NCH], f32, name="rch")
            ccA_in = dpool.tile([128, 4], f32)
            ccB_in = dpool.tile([128, 4], f32)
            ccA_out = nc.dram_tensor("ccA_out", [CORES, 128, 4], f32, addr_space="Shared")
            ccB_out = nc.dram_tensor("ccB_out", [CORES, 128, 4], f32, addr_space="Shared")

            def char_bank(pch, m, w0, nw):
                n = PW * nw
                a0 = PW * w0
                py = pch.tile([128, 512], f32, tag="py")
                nc.tensor.matmul(
                    out=py[:, :n],
                    lhsT=wdr_sb[:, m, :],
                    rhs=dr_rhs(x_sb, a0, n),
                    start=True,
                    stop=False,
                    perf_mode=mybir.MatmulPerfMode.DoubleRowSwInterleave,
                )
                nc.tensor.matmul(
                    out=py[:, :n],
                    lhsT=wn_sb[:, m, :],
                    rhs=x_sb[:, a0 + 2 : a0 + 2 + n],
                    start=False,
                    stop=True,
                )
                return py

            def drain_direct(py, m, w0, nw):
                pv = py[:, : PW * nw].rearrange("p (w t) -> p w t", t=PW)
                nc.vector.tensor_reduce(
                    out=u[:, 2 + m, w0 : w0 + nw],
                    in_=pv[:, :, 0:L],
                    axis=mybir.AxisListType.X,
                    op=mybir.AluOpType.max,
                )

            def drain_evac(py, strip, j, nw):
                pv = py[:, : PW * nw].rearrange("p (w t) -> p w t", t=PW)
                nc.scalar.activation(
                    out=strip[:, j, :nw, :],
                    in_=pv[:, :, 0:L],
                    func=mybir.ActivationFunctionType.Copy,
                )

            def tree(strip, m, w0, nwords):
                # strip [128, 4, 15, 32] holding nwords valid words
                sv = strip[:].rearrange("p b w t -> p (b w) t")[:, :nwords, :]
                l1 = tpool.tile([128, 60, 16], bf16, tag="l1", name="l1")
                l2 = tpool.tile([128, 60, 8], bf16, tag="l2", name="l2")
                l3 = tpool.tile([128, 60, 4], bf16, tag="l3", name="l3")
                l4 = tpool.tile([128, 60, 2], bf16, tag="l4", name="l4")
                nc.vector.tensor_tensor(
                    out=l1[:, :nwords, :], in0=sv[:, :, 0:16], in1=sv[:, :, 16:32],
                    op=mybir.AluOpType.max)
                nc.vector.tensor_tensor(
                    out=l2[:, :nwords, :], in0=l1[:, :nwords, 0:8],
                    in1=l1[:, :nwords, 8:16], op=mybir.AluOpType.max)
                nc.vector.tensor_tensor(
                    out=l3[:, :nwords, :], in0=l2[:, :nwords, 0:4],
                    in1=l2[:, :nwords, 4:8], op=mybir.AluOpType.max)
                nc.vector.tensor_tensor(
                    out=l4[:, :nwords, :], in0=l3[:, :nwords, 0:2],
                    in1=l3[:, :nwords, 2:4], op=mybir.AluOpType.max)
                nc.vector.tensor_tensor(
                    out=u[:, 2 + m, w0 : w0 + nwords], in0=l4[:, :nwords, 0],
                    in1=l4[:, :nwords, 1], op=mybir.AluOpType.max)

            def transposes(ptp, g):
                wcnt = min(128, NW - g * 128)
                for cc in range(2):
                    tp = ptp.tile([128, 128], f32, tag="tp", name="tp")
                    nc.tensor.transpose(
                        out=tp[:],
                        in_=wrd_sb[g][:, cc * 128 : (cc + 1) * 128],
                        identity=ident_sb[:],
                    )
                    nc.scalar.activation(
                        out=u[:, cc, g * 128 : g * 128 + wcnt],
                        in_=tp[:, :wcnt],
                        func=mybir.ActivationFunctionType.Copy,
                    )

            def sent_chunk(psn, c):
                # r[:, w] for w in [CH*c, CH*c+n): u cols [w, w+3)
                n = min(CH, WPC - c * CH)
                ps = psn.tile([128, 4, CH], f32, tag="ps", name="ps")
                for mb in range(4):
                    first = True
                    for k in range(3):
                        for kc in range(4):
                            nc.tensor.matmul(
                                out=ps[:, mb, :n],
                                lhsT=wsT_sb[:, k, kc, mb * 128 : (mb + 1) * 128],
                                rhs=u[:, kc, c * CH + k : c * CH + k + n],
                                start=first,
                                stop=(k == 2 and kc == 3),
                            )
                            first = False
                for mb in range(4):
                    nc.vector.tensor_reduce(
                        out=rch[:, mb, c : c + 1],
                        in_=ps[:, mb, :n],
                        axis=mybir.AxisListType.X,
                        op=mybir.AluOpType.max,
                    )

            # chunk c fires once u words [0, 128c+130) are done; with 75
            # words per supergroup that's after sg (128c+130)/75
            chunk_after_sg = {}
            for c in range(NCH - 1):
                chunk_after_sg.setdefault(min(-(-(CH * c + CH + 2) // 75), len(sgs)) - 1, []).append(c)

            # transpose group g must precede the sentence chunk that reads
            # its u-word cols; schedule at sg 0/1/2
            tp_at_sg = {0: [0, 1], 1: [2, 3], 2: [4]} if G == 5 else {0: list(range(G))}

            with nc.named_scope("char"):
                with (
                    tc.tile_pool(name="pch", bufs=5, space="PSUM") as pch,
                    tc.tile_pool(name="psn", bufs=1, space="PSUM") as psn,
                    tc.tile_pool(name="ptp", bufs=2, space="PSUM") as ptp,
                ):
                    for sgi, sg in enumerate(sgs):
                        strips = []
                        for m in range(2):
                            if len(sg) > 1:
                                strips.append(spool.tile([128, 4, WB, L], bf16, tag=f"s{m}", name=f"strip{m}"))
                            else:
                                strips.append(None)
                        for bi, (w0, nw) in enumerate(sg):
                            for m in range(2):
                                py = char_bank(pch, m, w0, nw)
                                if bi == 0:
                                    drain_direct(py, m, w0, nw)
                                else:
                                    drain_evac(py, strips[m], bi - 1, nw)
                            if bi == 0:
                                for g in tp_at_sg.get(sgi, []):
                                    transposes(ptp, g)
                        sg_w0 = sg[1][0] if len(sg) > 1 else None
                        sg_nw = sum(nw for (_, nw) in sg[1:])
                        for m in range(2):
                            if strips[m] is not None:
                                tree(strips[m], m, sg_w0, sg_nw)
                        for c in chunk_after_sg.get(sgi, []):
                            sent_chunk(psn, c)
                            if c == NCH - 2:
                                # phase A: max over chunks 0..NCH-2 (+ sent
                                # bias, so it needn't be added post-gather)
                                ra = cpool.tile([128, 4], f32)
                                nc.vector.tensor_reduce(
                                    out=ra[:],
                                    in_=rch[:, :, : NCH - 1],
                                    axis=mybir.AxisListType.X,
                                    op=mybir.AluOpType.max,
                                )
                                nc.vector.tensor_tensor(
                                    out=ra[:], in0=ra[:], in1=bsent_sb[:],
                                    op=mybir.AluOpType.add,
                                )
                                nc.gpsimd.dma_start(out=ccA_in[:], in_=ra[:])
                                nc.gpsimd.collective_compute(
                                    "AllGather",
                                    mybir.AluOpType.bypass,
                                    replica_groups=[list(range(CORES))],
                                    ins=[ccA_in[:]],
                                    outs=[ccA_out[:]],
                                )
                    # last sentence chunk + phase B (with bias folded in)
                    sent_chunk(psn, NCH - 1)
                    rb = cpool.tile([128, 4], f32)
                    nc.vector.tensor_tensor(
                        out=rb[:],
                        in0=rch[:, :, NCH - 1 : NCH].rearrange("p m c -> p (m c)"),
                        in1=bsent_sb[:],
                        op=mybir.AluOpType.add,
                    )
                    nc.gpsimd.dma_start(out=ccB_in[:], in_=rb[:])
                    nc.gpsimd.collective_compute(
                        "AllGather",
                        mybir.AluOpType.bypass,
                        replica_groups=[list(range(CORES))],
                        ins=[ccB_in[:]],
                        outs=[ccB_out[:]],
                    )

            # ---- global max + MLP -----------------------------------------
            rg = cpool.tile([128, 2, CORES, 4], f32)
            nc.sync.dma_start(out=rg[:, 0], in_=ccA_out[:].rearrange("r p f -> p r f"))
            nc.sync.dma_start(out=rg[:, 1], in_=ccB_out[:].rearrange("r p f -> p r f"))
            r_sb = cpool.tile([128, 4], bf16)
            nc.vector.tensor_reduce(
                out=r_sb[:],
                in_=rg[:].rearrange("p g r f -> p f g r"),
                axis=mybir.AxisListType.XY,
                op=mybir.AluOpType.max,
            )
            with tc.tile_pool(name="pmlp", bufs=2, space="PSUM") as pmlp:
                hp = pmlp.tile([128, 8], f32, tag="hp")
                for j in range(8):
                    for k in range(4):
                        nc.tensor.matmul(
                            out=hp[:, j : j + 1],
                            lhsT=w1t_sb[:, k, j, :],
                            rhs=r_sb[:, k : k + 1],
                            start=(k == 0),
                            stop=(k == 3),
                        )
                hb = cpool.tile([128, 8], f32)
                nc.vector.tensor_tensor(
                    out=hb[:], in0=hp[:], in1=b1t_sb[:], op=mybir.AluOpType.add
                )
                h_sb = cpool.tile([128, 8], bf16)
                nc.scalar.activation(
                    out=h_sb[:], in_=hb[:], func=mybir.ActivationFunctionType.Tanh
                )
                o_ps = pmlp.tile([2, 1], f32, tag="op")
                for k in range(8):
                    nc.tensor.matmul(
                        out=o_ps[:],
                        lhsT=w2t_sb[:, k, :],
                        rhs=h_sb[:, k : k + 1],
                        start=(k == 0),
                        stop=(k == 7),
                    )
                o_sb = cpool.tile([2, 1], f32)
                nc.vector.tensor_tensor(
                    out=o_sb[:], in0=o_ps[:], in1=b2t_sb[:], op=mybir.AluOpType.add
                )
                nc.sync.dma_start(out=out[:], in_=o_sb[:])

    nc.finalize()
    return nc


def prep_in_maps(words, words_in_char, word_emb, chr_emb, conv_chr_w, conv_chr_b,
                 conv_sent_w, conv_sent_b, w1, b1, w2, b2):
    W = words.shape[0]
    WPC, NW, SX, SXP, NB, G = _shapes(W)

    words = np.asarray(words, np.int32)
    chars = np.asarray(words_in_char, np.int32)
    word_emb = np.asarray(word_emb, np.float32)
    chr_emb = np.asarray(chr_emb, np.float32)
    conv_chr_w = np.asarray(conv_chr_w, np.float32)
    conv_chr_b = np.asarray(conv_chr_b, np.float32)
    conv_sent_w = np.asarray(conv_sent_w, np.float32)
    conv_sent_b = np.asarray(conv_sent_b, np.float32)
    w1 = np.asarray(w1, np.float32)
    b1 = np.asarray(b1, np.float32)
    w2 = np.asarray(w2, np.float32)
    b2 = np.asarray(b2, np.float32)

    # host-folded char response tables (x64 into fp8 range); conv bias is
    # folded into the center tap's table (center col is never a pad col)
    ET = [ETS * (chr_emb @ conv_chr_w[:, :, k].T) for k in range(3)]
    ET[1] = ET[1] + ETS * conv_chr_b[None, :]
    ET = [t.astype(FP8).astype(np.float32) for t in ET]

    # DR SWI weights for (ET0, ET1'), per m-half; normal weights = ET2
    wdr = np.zeros((128, 2, 256), np.float32)
    wn = np.zeros((128, 2, 128), np.float32)
    for m in range(2):
        a = ET[0][:, m * 128 : (m + 1) * 128]
        b = ET[1][:, m * 128 : (m + 1) * 128]
        wdr[:, m, :] = np.stack([a[:, ::-1], b[:, ::-1]], axis=2).reshape(128, 256)
        wn[:, m, :] = ET[2][:, m * 128 : (m + 1) * 128]

    # sentence conv: char-half input channels absorb the 1/64
    ws = conv_sent_w.copy()
    ws[:, D:, :] /= ETS
    wsT = np.ascontiguousarray(
        ws.transpose(1, 2, 0).reshape(4, 128, 3, 2 * D).transpose(1, 2, 0, 3)
    ).astype(BF16)                                        # [p, k, kc, c2]
    bsent = np.ascontiguousarray(conv_sent_b.reshape(4, 128).T).astype(np.float32)
    w1t = np.ascontiguousarray(
        w1.reshape(8, 128, 4, 128).transpose(3, 2, 0, 1)
    ).astype(BF16)                                  # [p, k, m, c]
    b1t = np.ascontiguousarray(b1.reshape(8, 128).T).astype(np.float32)
    w2t = np.ascontiguousarray(
        w2.T.reshape(8, 128, 2).transpose(1, 0, 2)
    ).astype(BF16)                                  # [p, k, j]
    b2t = b2.reshape(2, 1).astype(np.float32)
    ident = np.eye(128, dtype=np.float32)

    in_maps = []
    for c in range(CORES):
        lo = c * WPC - 1
        idxs = np.arange(lo, lo + NW)
        valid = (idxs >= 0) & (idxs < W)
        w_ext = np.where(valid, words[np.clip(idxs, 0, W - 1)], 0).astype(np.int32)
        ch_ext = np.zeros((NW, L), np.int32)
        ch_ext[valid] = chars[np.clip(idxs, 0, W - 1)[valid]]

        # one-hot stream: word w chars at cols PW*w+1 .. PW*w+32; halo words
        # outside [0, W) stay all-zero (their u must be exactly 0)
        oh = np.zeros((128, SXP), FP8)
        cols = (PW * np.arange(NW)[:, None] + 1 + np.arange(L)[None, :])
        vmask = np.repeat(valid, L)
        cf = cols.reshape(-1)[vmask]
        chf = ch_ext.reshape(-1)[vmask]
        oh[chf, cf] = 1.0

        wpad = np.zeros(G * 128, np.int32)
        wpad[:NW] = w_ext
        widx = np.ascontiguousarray(wpad.reshape(G, 128).T)

        in_maps.append(
            dict(
                onehot=oh,
                wdr=wdr.astype(FP8),
                wn=wn.astype(FP8),
                widx=widx,
                wemb=word_emb,
                wsT=wsT,
                bsent=bsent,
                w1t=w1t,
                b1t=b1t,
                w2t=w2t,
                b2t=b2t,
                ident=ident,
            )
        )
    return in_maps


_CACHE = {}


def _get_nc(W):
    if W not in _CACHE:
        _CACHE[W] = build(W)
    return _CACHE[W]


def run(inputs, trace=False):
    W = np.asarray(inputs["words"]).shape[0]
    nc = _get_nc(W)
    in_maps = prep_in_maps(**inputs)
    res = run_bass_kernel_spmd(nc, in_maps, list(range(CORES)), trace=trace)
    out = np.asarray(res.results[0]["out"], np.float32).reshape(1, 2)
    return out, res


def kernel(**inputs) -> np.ndarray:
    out, _ = run(inputs, trace=False)
    return out


# revision 15
# speedup vs baseline: 1.1666x; 1.0277x over previous
"""Trainium2 Bass kernel for nn_ConvNet (char-CNN word encoder + sentence conv + MLP).

Single-stream char path: one one-hot stream X[128 vocab, SX] fp8 with one
zero pad column between words (33 cols/word).  The 3-tap conv collapses to
shifted one-hot matmuls into the same PSUM bank:
    out[j] = ET0.X[j] + ET1'.X[j+1] + ET2.X[j+2]
where ET1' has the conv bias folded in (center tap is never a pad col for a
real output).  The DoubleRow fp8 pass computes (ET0, ET1') via an
OVERLAPPING rhs AP (slots read X[j], X[j+1] from the same stream); the
third tap is a normal fp8 matmul on X[j+2:].  Word isolation is automatic:
pad cols contribute exactly 0, and outputs at pad positions (j%33==32) are
garbage that the strided max simply skips.

PSUM drain (the DVE bottleneck in v1) is split: per 5-bank supergroup, one
bank is max-reduced directly from PSUM by the DVE; four banks are evacuated
by the Scalar engine (strided compact copy to bf16 SBUF) and reduced by a
DVE tensor_tensor max tree, which runs in the 2x_1p DVE mode (2 elem/cyc)
unlike tensor_reduce (1x only).

The sentence conv (bf16, 48 matmuls) is interleaved into the char phase in
4 chunks of 128 words so the PE never idles (p-state stays at 2.4 GHz) and
only the last chunk is tail-exposed.  The 8-way max uses two AllGathers of
[128,4] partial maxes: phase A (chunks 0-2) launches ~85% through the char
phase and hides its latency; phase B (chunk 3) pays only its own sync.
"""

import sys

try:
    import concourse  # noqa: F401
except ImportError:
    sys.path.insert(0, "/opt/trn_rl_repo")

import numpy as np
import ml_dtypes

import concourse.bass as bass
import concourse.bacc as bacc
import concourse.tile as tile
from concourse import mybir
from concourse.bass_utils import run_bass_kernel_spmd
from bass_rust import VecI64Pair

BF16 = ml_dtypes.bfloat16
FP8 = ml_dtypes.float8_e4m3

CORES = 8
D = 256
L = 32
PW = L + 1          # stream cols per word (32 chars + 1 pad)
WB = 15             # words per PSUM bank (15*33 = 495 <= 512)
ETS = 64.0          # fp8 scale for the ET response tables


def _shapes(W):
    WPC = W // CORES            # real words per core
    NW = WPC + 2                # + 1 halo word each side
    SX = PW * NW + 1 + 2        # stream cols (+lead pad, +2 tail zeros)
    SXP = -(-SX // 16) * 16     # padded to 16
    NB = -(-NW // WB)           # banks per m-half
    G = -(-NW // 128)           # word-gather groups of 128
    return WPC, NW, SX, SXP, NB, G


def build(W):
    WPC, NW, SX, SXP, NB, G = _shapes(W)
    f32 = mybir.dt.float32
    bf16 = mybir.dt.bfloat16
    f8 = mybir.dt.float8e4
    i32 = mybir.dt.int32

    # banks: (word0, nwords); supergroups of 5 banks (last ragged)
    banks = []
    w0 = 0
    while w0 < NW:
        banks.append((w0, min(WB, NW - w0)))
        w0 += WB
    sgs = [banks[i : i + 5] for i in range(0, len(banks), 5)]

    # sentence chunks of ~WPC/3 real words; chunk c ready after u words
    # [0, CH(c+1)+2) exist
    CH = 128
    NCH = -(-WPC // CH)
    UPAD = -(-NW // 16) * 16

    nc = bacc.Bacc(num_devices=CORES)

    onehot = nc.declare_dram_parameter("onehot", [128, SXP], f8, isOutput=False)
    wdr = nc.declare_dram_parameter("wdr", [128, 2, 256], f8, isOutput=False)
    wn = nc.declare_dram_parameter("wn", [128, 2, 128], f8, isOutput=False)
    widx = nc.declare_dram_parameter("widx", [128, G], i32, isOutput=False)
    wemb = nc.declare_dram_parameter("wemb", [50000, D], f32, isOutput=False)
    wsT = nc.declare_dram_parameter("wsT", [128, 3, 4, 2 * D], bf16, isOutput=False)
    bsent = nc.declare_dram_parameter("bsent", [128, 4], f32, isOutput=False)
    w1t = nc.declare_dram_parameter("w1t", [128, 4, 8, 128], bf16, isOutput=False)
    b1t = nc.declare_dram_parameter("b1t", [128, 8], f32, isOutput=False)
    w2t = nc.declare_dram_parameter("w2t", [128, 8, 2], bf16, isOutput=False)
    b2t = nc.declare_dram_parameter("b2t", [2, 1], f32, isOutput=False)
    ident = nc.declare_dram_parameter("ident", [128, 128], f32, isOutput=False)
    out = nc.declare_dram_parameter("out", [2, 1], f32, isOutput=True)

    def dr_rhs(x_sb, a0, n):
        # overlapping DoubleRow rhs: slots read X[:, a0+j] and X[:, a0+j+1]
        v = x_sb[:, a0 : a0 + n].unsqueeze(1).broadcast_to([128, 2, n])
        v.ap = VecI64Pair([[SXP, 128], [1, 2], [1, n]])
        return v

    with tile.TileContext(nc) as tc:
        with (
            tc.tile_pool(name="const", bufs=1) as cpool,
            tc.tile_pool(name="gath", bufs=G) as gpool,
            tc.tile_pool(name="strip", bufs=4) as spool,
            tc.tile_pool(name="tree", bufs=2) as tpool,
            tc.tile_pool(name="dram", bufs=1, space="DRAM") as dpool,
        ):
            x_sb = cpool.tile([128, SXP], f8, name="x_sb")
            wdr_sb = cpool.tile([128, 2, 256], f8)
            wn_sb = cpool.tile([128, 2, 128], f8)
            widx_sb = cpool.tile([128, G], i32)
            wsT_sb = cpool.tile([128, 3, 4, 2 * D], bf16)
            ident_sb = cpool.tile([128, 128], f32)
            bsent_sb = cpool.tile([128, 4], f32)
            w1t_sb = cpool.tile([128, 4, 8, 128], bf16)
            b1t_sb = cpool.tile([128, 8], f32)
            w2t_sb = cpool.tile([128, 8, 2], bf16)
            b2t_sb = cpool.tile([2, 1], f32)
            u = cpool.tile([128, 4, UPAD], bf16, name="u")

            dum_in = dpool.tile([128, 2], f32)
            dum_out = nc.dram_tensor("dum_out", [CORES, 128, 2], f32, addr_space="Shared")

            # ---- head DMAs -------------------------------------------------
            # weights first (tiny) so bank-0 matmuls can start immediately
            nc.scalar.dma_start(out=wdr_sb[:], in_=wdr[:])
            nc.scalar.dma_start(out=wn_sb[:], in_=wn[:])
            # big sentence weights: one async transfer issued early
            nc.scalar.dma_start(out=wsT_sb[:], in_=wsT[:])
            # widx first so the word-emb gathers can start early
            nc.gpsimd.dma_start(out=widx_sb[:], in_=widx[:])
            # one-hot stream sections, alternating sync/gpsimd; first small
            secs = [0, 2 * 495 + 2]
            s = secs[-1]
            step = -(-(SXP - s) // 6)
            while s < SXP:
                s = min(s + step, SXP)
                secs.append(s)
            wrd_sb = [gpool.tile([128, D], f32, tag="wrd", name=f"wrd{g}") for g in range(G)]

            def gather(g):
                nc.gpsimd.indirect_dma_start(
                    out=wrd_sb[g][:],
                    out_offset=None,
                    in_=wemb[:],
                    in_offset=bass.IndirectOffsetOnAxis(ap=widx_sb[:, g : g + 1], axis=0),
                )

            gather(0)
            gather(1)
            for si, (a, b) in enumerate(zip(secs[:-1], secs[1:])):
                eng = nc.sync if si % 2 == 0 else nc.gpsimd
                eng.dma_start(out=x_sb[:, a:b], in_=onehot[:, a:b])
                if si == 1:
                    gather(2)
                if si == 3:
                    gather(3)
                    gather(4)
            nc.sync.dma_start(out=ident_sb[:], in_=ident[:])
            # warm-up collective: arms ncfw so the real AllGathers are cheap
            nc.gpsimd.dma_start(out=dum_in[:], in_=widx_sb[:].bitcast(f32)[:, 0:2])
            nc.gpsimd.collective_compute(
                "AllGather",
                mybir.AluOpType.bypass,
                replica_groups=[list(range(CORES))],
                ins=[dum_in[:]],
                outs=[dum_out[:]],
            )
            nc.gpsimd.dma_start(out=bsent_sb[:], in_=bsent[:])
            nc.gpsimd.dma_start(out=b1t_sb[:], in_=b1t[:])
            nc.gpsimd.dma_start(out=b2t_sb[:], in_=b2t[:])
            nc.gpsimd.dma_start(out=w1t_sb[:], in_=w1t[:])
            nc.gpsimd.dma_start(out=w2t_sb[:], in_=w2t[:])

            # ---- char pipeline + interleaved sentence conv -----------------
            rch = cpool.tile([128, 4, NCH], f32, name="rch")
            ccA_in = dpool.tile([128, 4], f32)
            ccB_in = dpool.tile([128, 4], f32)
            ccA_out = nc.dram_tensor("ccA_out", [CORES, 128, 4], f32, addr_space="Shared")
            ccB_out = nc.dram_tensor("ccB_out", [CORES, 128, 4], f32, addr_space="Shared")

            def char_bank(pch, m, w0, nw):
                n = PW * nw
                a0 = PW * w0
                py = pch.tile([128, 512], f32, tag="py")
                nc.tensor.matmul(
                    out=py[:, :n],
                    lhsT=wdr_sb[:, m, :],
                    rhs=dr_rhs(x_sb, a0, n),
                    start=True,
                    stop=False,
                    perf_mode=mybir.MatmulPerfMode.DoubleRowSwInterleave,
                )
                nc.tensor.matmul(
                    out=py[:, :n],
                    lhsT=wn_sb[:, m, :],
                    rhs=x_sb[:, a0 + 2 : a0 + 2 + n],
                    start=False,
                    stop=True,
                )
                return py

            def drain_direct(py, m, w0, nw):
                pv = py[:, : PW * nw].rearrange("p (w t) -> p w t", t=PW)
                nc.vector.tensor_reduce(
                    out=u[:, 2 + m, w0 : w0 + nw],
                    in_=pv[:, :, 0:L],
                    axis=mybir.AxisListType.X,
                    op=mybir.AluOpType.max,
                )

            def drain_evac(py, strip, j, nw):
                pv = py[:, : PW * nw].rearrange("p (w t) -> p w t", t=PW)
                nc.scalar.activation(
                    out=strip[:, j, :nw, :],
                    in_=pv[:, :, 0:L],
                    func=mybir.ActivationFunctionType.Copy,
                )

            def tree(strip, m, w0, nwords):
                # strip [128, 4, 15, 32] holding nwords valid words
                sv = strip[:].rearrange("p b w t -> p (b w) t")[:, :nwords, :]
                l1 = tpool.tile([128, 60, 16], bf16, tag="l1", name="l1")
                l2 = tpool.tile([128, 60, 8], bf16, tag="l2", name="l2")
                l3 = tpool.tile([128, 60, 4], bf16, tag="l3", name="l3")
                l4 = tpool.tile([128, 60, 2], bf16, tag="l4", name="l4")
                nc.vector.tensor_tensor(
                    out=l1[:, :nwords, :], in0=sv[:, :, 0:16], in1=sv[:, :, 16:32],
                    op=mybir.AluOpType.max)
                nc.vector.tensor_tensor(
                    out=l2[:, :nwords, :], in0=l1[:, :nwords, 0:8],
                    in1=l1[:, :nwords, 8:16], op=mybir.AluOpType.max)
                nc.vector.tensor_tensor(
                    out=l3[:, :nwords, :], in0=l2[:, :nwords, 0:4],
                    in1=l2[:, :nwords, 4:8], op=mybir.AluOpType.max)
                nc.vector.tensor_tensor(
                    out=l4[:, :nwords, :], in0=l3[:, :nwords, 0:2],
                    in1=l3[:, :nwords, 2:4], op=mybir.AluOpType.max)
                nc.vector.tensor_tensor(
                    out=u[:, 2 + m, w0 : w0 + nwords], in0=l4[:, :nwords, 0],
                    in1=l4[:, :nwords, 1], op=mybir.AluOpType.max)

            def transposes(ptp, g):
                wcnt = min(128, NW - g * 128)
                for cc in range(2):
                    tp = ptp.tile([128, 128], f32, tag="tp", name="tp")
                    nc.tensor.transpose(
                        out=tp[:],
                        in_=wrd_sb[g][:, cc * 128 : (cc + 1) * 128],
                        identity=ident_sb[:],
                    )
                    nc.scalar.activation(
                        out=u[:, cc, g * 128 : g * 128 + wcnt],
                        in_=tp[:, :wcnt],
                        func=mybir.ActivationFunctionType.Copy,
                    )

            def sent_mms(psn, c):
                # 48 matmul closures for sentence chunk c (woven between char
                # banks so their LDWEIGHTS hide under big char matmuls)
                n = min(CH, WPC - c * CH)
                ps = psn.tile([128, 4, CH], f32, tag="ps", name="ps")
                items = []
                for mb in range(4):
                    for ki in range(12):
                        k, kc = divmod(ki, 4)
                        items.append((ps, mb, k, kc, n, ki == 0, ki == 11))
                return ps, n, items

            def emit_sent_mm(c, it):
                ps, mb, k, kc, n, first, last = it
                nc.tensor.matmul(
                    out=ps[:, mb, :n],
                    lhsT=wsT_sb[:, k, kc, mb * 128 : (mb + 1) * 128],
                    rhs=u[:, kc, c * CH + k : c * CH + k + n],
                    start=first,
                    stop=last,
                )

            def sent_reduce(ps, c, n):
                for mb in range(4):
                    nc.vector.tensor_reduce(
                        out=rch[:, mb, c : c + 1],
                        in_=ps[:, mb, :n],
                        axis=mybir.AxisListType.X,
                        op=mybir.AluOpType.max,
                    )

            def phase_a():
                # max over chunks 0..NCH-2 (+ sentence bias, folded here so
                # it needn't be added post-gather)
                ra = cpool.tile([128, 4], f32)
                nc.vector.tensor_reduce(
                    out=ra[:],
                    in_=rch[:, :, : NCH - 1],
                    axis=mybir.AxisListType.X,
                    op=mybir.AluOpType.max,
                )
                nc.vector.tensor_tensor(
                    out=ra[:], in0=ra[:], in1=bsent_sb[:], op=mybir.AluOpType.add
                )
                nc.gpsimd.dma_start(out=ccA_in[:], in_=ra[:])
                nc.gpsimd.collective_compute(
                    "AllGather",
                    mybir.AluOpType.bypass,
                    replica_groups=[list(range(CORES))],
                    ins=[ccA_in[:]],
                    outs=[ccA_out[:]],
                )

            # chunk c fires once u words [0, 128c+130) are done; with 75
            # words per supergroup that's after sg (128c+130)/75
            # transpose group g must precede the sentence chunk that reads
            # its u-word cols; schedule at sg 0/1/2
            tp_at_sg = {0: [0, 1], 1: [2, 3], 2: [4]} if G == 5 else {0: list(range(G))}
            # chunk c weaves into sg w(c): its u-deps (words < 128c+130) are
            # emitted by then (trees of sg w-1 emit at sg w after bank 0, and
            # sg w's own bank-0 direct reduce covers the boundary words)
            weave_at_sg = {2: 0, 4: 1, 5: 2}

            rg = cpool.tile([128, 2, CORES, 4], f32, name="rg")

            with nc.named_scope("char"):
                with (
                    tc.tile_pool(name="pch", bufs=5, space="PSUM") as pch,
                    tc.tile_pool(name="psn", bufs=1, space="PSUM") as psn,
                    tc.tile_pool(name="ptp", bufs=2, space="PSUM") as ptp,
                ):
                    pending_trees = []
                    for sgi, sg in enumerate(sgs):
                        strips = []
                        for m in range(2):
                            if len(sg) > 1:
                                strips.append(spool.tile([128, 4, WB, L], bf16, tag=f"s{m}", name=f"strip{m}"))
                            else:
                                strips.append(None)
                        wv_c = weave_at_sg.get(sgi)
                        wv_items = []
                        if wv_c is not None:
                            wv_ps, wv_n, wv_items = sent_mms(psn, wv_c)
                        for bi, (w0, nw) in enumerate(sg):
                            for m in range(2):
                                py = char_bank(pch, m, w0, nw)
                                if bi == 0:
                                    drain_direct(py, m, w0, nw)
                                else:
                                    drain_evac(py, strips[m], bi - 1, nw)
                            if bi == 0:
                                # delayed trees: prior sg's strips are fully
                                # evacuated by now; emitting them here keeps
                                # the DVE queue from head-of-line blocking
                                for t in pending_trees:
                                    tree(*t)
                                pending_trees = []
                                for g in tp_at_sg.get(sgi, []):
                                    transposes(ptp, g)
                            elif wv_items:
                                # weave sentence matmuls between char banks
                                take = -(-len(wv_items) // ((5 - bi) * 2 - 1))
                                for _ in range(take):
                                    emit_sent_mm(wv_c, wv_items.pop(0))
                        if wv_c is not None:
                            for it in wv_items:
                                emit_sent_mm(wv_c, it)
                            sent_reduce(wv_ps, wv_c, wv_n)
                            if wv_c == NCH - 2:
                                phase_a()
                                nc.sync.dma_start(
                                    out=rg[:, 0], in_=ccA_out[:].rearrange("r p f -> p r f")
                                )
                        sg_w0 = sg[1][0] if len(sg) > 1 else None
                        sg_nw = sum(nw for (_, nw) in sg[1:])
                        for m in range(2):
                            if strips[m] is not None:
                                pending_trees.append((strips[m], m, sg_w0, sg_nw))
                    for t in pending_trees:
                        tree(*t)
                    # last sentence chunk (tail) + phase B (bias folded in)
                    ps3, n3, items3 = sent_mms(psn, NCH - 1)
                    for it in items3:
                        emit_sent_mm(NCH - 1, it)
                    sent_reduce(ps3, NCH - 1, n3)
                    rb = cpool.tile([128, 4], f32)
                    nc.vector.tensor_tensor(
                        out=rb[:],
                        in0=rch[:, :, NCH - 1 : NCH].rearrange("p m c -> p (m c)"),
                        in1=bsent_sb[:],
                        op=mybir.AluOpType.add,
                    )
                    nc.gpsimd.dma_start(out=ccB_in[:], in_=rb[:])
                    nc.gpsimd.collective_compute(
                        "AllGather",
                        mybir.AluOpType.bypass,
                        replica_groups=[list(range(CORES))],
                        ins=[ccB_in[:]],
                        outs=[ccB_out[:]],
                    )

            # ---- global max + MLP -----------------------------------------
            nc.sync.dma_start(out=rg[:, 1], in_=ccB_out[:].rearrange("r p f -> p r f"))
            r_sb = cpool.tile([128, 4], bf16)
            nc.vector.tensor_reduce(
                out=r_sb[:],
                in_=rg[:].rearrange("p g r f -> p f g r"),
                axis=mybir.AxisListType.XY,
                op=mybir.AluOpType.max,
            )
            with tc.tile_pool(name="pmlp", bufs=2, space="PSUM") as pmlp:
                hp = pmlp.tile([128, 8], f32, tag="hp")
                for j in range(8):
                    for k in range(4):
                        nc.tensor.matmul(
                            out=hp[:, j : j + 1],
                            lhsT=w1t_sb[:, k, j, :],
                            rhs=r_sb[:, k : k + 1],
                            start=(k == 0),
                            stop=(k == 3),
                        )
                hb = cpool.tile([128, 8], f32)
                nc.vector.tensor_tensor(
                    out=hb[:], in0=hp[:], in1=b1t_sb[:], op=mybir.AluOpType.add
                )
                h_sb = cpool.tile([128, 8], bf16)
                nc.scalar.activation(
                    out=h_sb[:], in_=hb[:], func=mybir.ActivationFunctionType.Tanh
                )
                o_ps = pmlp.tile([2, 1], f32, tag="op")
                for k in range(8):
                    nc.tensor.matmul(
                        out=o_ps[:],
                        lhsT=w2t_sb[:, k, :],
                        rhs=h_sb[:, k : k + 1],
                        start=(k == 0),
                        stop=(k == 7),
                    )
                o_sb = cpool.tile([2, 1], f32)
                nc.vector.tensor_tensor(
                    out=o_sb[:], in0=o_ps[:], in1=b2t_sb[:], op=mybir.AluOpType.add
                )
                nc.sync.dma_start(out=out[:], in_=o_sb[:])

    nc.finalize()
    return nc


def prep_in_maps(words, words_in_char, word_emb, chr_emb, conv_chr_w, conv_chr_b,
                 conv_sent_w, conv_sent_b, w1, b1, w2, b2):
    W = words.shape[0]
    WPC, NW, SX, SXP, NB, G = _shapes(W)

    words = np.asarray(words, np.int32)
    chars = np.asarray(words_in_char, np.int32)
    word_emb = np.asarray(word_emb, np.float32)
    chr_emb = np.asarray(chr_emb, np.float32)
    conv_chr_w = np.asarray(conv_chr_w, np.float32)
    conv_chr_b = np.asarray(conv_chr_b, np.float32)
    conv_sent_w = np.asarray(conv_sent_w, np.float32)
    conv_sent_b = np.asarray(conv_sent_b, np.float32)
    w1 = np.asarray(w1, np.float32)
    b1 = np.asarray(b1, np.float32)
    w2 = np.asarray(w2, np.float32)
    b2 = np.asarray(b2, np.float32)

    # host-folded char response tables (x64 into fp8 range); conv bias is
    # folded into the center tap's table (center col is never a pad col)
    ET = [ETS * (chr_emb @ conv_chr_w[:, :, k].T) for k in range(3)]
    ET[1] = ET[1] + ETS * conv_chr_b[None, :]
    ET = [t.astype(FP8).astype(np.float32) for t in ET]

    # DR SWI weights for (ET0, ET1'), per m-half; normal weights = ET2
    wdr = np.zeros((128, 2, 256), np.float32)
    wn = np.zeros((128, 2, 128), np.float32)
    for m in range(2):
        a = ET[0][:, m * 128 : (m + 1) * 128]
        b = ET[1][:, m * 128 : (m + 1) * 128]
        wdr[:, m, :] = np.stack([a[:, ::-1], b[:, ::-1]], axis=2).reshape(128, 256)
        wn[:, m, :] = ET[2][:, m * 128 : (m + 1) * 128]

    # sentence conv: char-half input channels absorb the 1/64
    ws = conv_sent_w.copy()
    ws[:, D:, :] /= ETS
    wsT = np.ascontiguousarray(
        ws.transpose(1, 2, 0).reshape(4, 128, 3, 2 * D).transpose(1, 2, 0, 3)
    ).astype(BF16)                                        # [p, k, kc, c2]
    bsent = np.ascontiguousarray(conv_sent_b.reshape(4, 128).T).astype(np.float32)
    w1t = np.ascontiguousarray(
        w1.reshape(8, 128, 4, 128).transpose(3, 2, 0, 1)
    ).astype(BF16)                                  # [p, k, m, c]
    b1t = np.ascontiguousarray(b1.reshape(8, 128).T).astype(np.float32)
    w2t = np.ascontiguousarray(
        w2.T.reshape(8, 128, 2).transpose(1, 0, 2)
    ).astype(BF16)                                  # [p, k, j]
    b2t = b2.reshape(2, 1).astype(np.float32)
    ident = np.eye(128, dtype=np.float32)

    in_maps = []
    for c in range(CORES):
        lo = c * WPC - 1
        idxs = np.arange(lo, lo + NW)
        valid = (idxs >= 0) & (idxs < W)
        w_ext = np.where(valid, words[np.clip(idxs, 0, W - 1)], 0).astype(np.int32)
        ch_ext = np.zeros((NW, L), np.int32)
        ch_ext[valid] = chars[np.clip(idxs, 0, W - 1)[valid]]

        # one-hot stream: word w chars at cols PW*w+1 .. PW*w+32; halo words
        # outside [0, W) stay all-zero (their u must be exactly 0)
        oh = np.zeros((128, SXP), FP8)
        cols = (PW * np.arange(NW)[:, None] + 1 + np.arange(L)[None, :])
        vmask = np.repeat(valid, L)
        cf = cols.reshape(-1)[vmask]
        chf = ch_ext.reshape(-1)[vmask]
        oh[chf, cf] = 1.0

        wpad = np.zeros(G * 128, np.int32)
        wpad[:NW] = w_ext
        widx = np.ascontiguousarray(wpad.reshape(G, 128).T)

        in_maps.append(
            dict(
                onehot=oh,
                wdr=wdr.astype(FP8),
                wn=wn.astype(FP8),
                widx=widx,
                wemb=word_emb,
                wsT=wsT,
                bsent=bsent,
                w1t=w1t,
                b1t=b1t,
                w2t=w2t,
                b2t=b2t,
                ident=ident,
            )
        )
    return in_maps


_CACHE = {}


def _get_nc(W):
    if W not in _CACHE:
        _CACHE[W] = build(W)
    return _CACHE[W]


def run(inputs, trace=False):
    W = np.asarray(inputs["words"]).shape[0]
    nc = _get_nc(W)
    in_maps = prep_in_maps(**inputs)
    res = run_bass_kernel_spmd(nc, in_maps, list(range(CORES)), trace=trace)
    out = np.asarray(res.results[0]["out"], np.float32).reshape(1, 2)
    return out, res


def kernel(**inputs) -> np.ndarray:
    out, _ = run(inputs, trace=False)
    return out


# revision 17
# speedup vs baseline: 1.2507x; 1.0721x over previous
"""Trainium2 Bass kernel for nn_ConvNet (char-CNN word encoder + sentence conv + MLP).

Single-stream char path: one one-hot stream X[128 vocab, SX] fp8 with one
zero pad column between words (33 cols/word).  The 3-tap conv collapses to
shifted one-hot matmuls into the same PSUM bank:
    out[j] = ET0.X[j] + ET1'.X[j+1] + ET2.X[j+2]
where ET1' has the conv bias folded in (center tap is never a pad col for a
real output).  The DoubleRow fp8 pass computes (ET0, ET1') via an
OVERLAPPING rhs AP (slots read X[j], X[j+1] from the same stream); the
third tap is a normal fp8 matmul on X[j+2:].  Word isolation is automatic:
pad cols contribute exactly 0, and outputs at pad positions (j%33==32) are
garbage that the strided max simply skips.

PSUM drain (the DVE bottleneck in v1) is split: per 5-bank supergroup, one
bank is max-reduced directly from PSUM by the DVE; four banks are evacuated
by the Scalar engine (strided compact copy to bf16 SBUF) and reduced by a
DVE tensor_tensor max tree, which runs in the 2x_1p DVE mode (2 elem/cyc)
unlike tensor_reduce (1x only).

The sentence conv (bf16, 48 matmuls) is interleaved into the char phase in
4 chunks of 128 words so the PE never idles (p-state stays at 2.4 GHz) and
only the last chunk is tail-exposed.  The 8-way max uses two AllGathers of
[128,4] partial maxes: phase A (chunks 0-2) launches ~85% through the char
phase and hides its latency; phase B (chunk 3) pays only its own sync.
"""

import sys

try:
    import concourse  # noqa: F401
except ImportError:
    sys.path.insert(0, "/opt/trn_rl_repo")

import numpy as np
import ml_dtypes

import concourse.bass as bass
import concourse.bacc as bacc
import concourse.tile as tile
from concourse import mybir
from concourse.bass_utils import run_bass_kernel_spmd
from bass_rust import VecI64Pair

BF16 = ml_dtypes.bfloat16
FP8 = ml_dtypes.float8_e4m3

CORES = 8
D = 256
L = 32
PW = L + 1          # stream cols per word (32 chars + 1 pad)
WB = 15             # words per PSUM bank (15*33 = 495 <= 512)
ETS = 64.0          # fp8 scale for the ET response tables


def _shapes(W):
    WPC = W // CORES            # real words per core
    NW = WPC + 2                # + 1 halo word each side
    SX = PW * NW + 1 + 2        # stream cols (+lead pad, +2 tail zeros)
    SXP = -(-SX // 16) * 16     # padded to 16
    NB = -(-NW // WB)           # banks per m-half
    G = -(-NW // 128)           # word-gather groups of 128
    return WPC, NW, SX, SXP, NB, G


def build(W):
    WPC, NW, SX, SXP, NB, G = _shapes(W)
    f32 = mybir.dt.float32
    bf16 = mybir.dt.bfloat16
    f8 = mybir.dt.float8e4
    i32 = mybir.dt.int32

    # banks: (word0, nwords); supergroups of 5 banks (last ragged)
    banks = []
    w0 = 0
    while w0 < NW:
        banks.append((w0, min(WB, NW - w0)))
        w0 += WB
    sgs = [banks[i : i + 5] for i in range(0, len(banks), 5)]

    # sentence chunks of ~WPC/3 real words; chunk c ready after u words
    # [0, CH(c+1)+2) exist
    CH = 128
    NCH = -(-WPC // CH)
    UPAD = -(-NW // 16) * 16

    nc = bacc.Bacc(num_devices=CORES)

    onehot = nc.declare_dram_parameter("onehot", [128, SXP], f8, isOutput=False)
    wdr = nc.declare_dram_parameter("wdr", [128, 2, 256], f8, isOutput=False)
    wn = nc.declare_dram_parameter("wn", [128, 2, 128], f8, isOutput=False)
    widx = nc.declare_dram_parameter("widx", [128, G], i32, isOutput=False)
    wemb = nc.declare_dram_parameter("wemb", [50000, D], f32, isOutput=False)
    wsT = nc.declare_dram_parameter("wsT", [128, 3, 4, 2 * D], bf16, isOutput=False)
    bsent = nc.declare_dram_parameter("bsent", [128, 4], f32, isOutput=False)
    w1t = nc.declare_dram_parameter("w1t", [128, 4, 8, 128], bf16, isOutput=False)
    b1t = nc.declare_dram_parameter("b1t", [128, 8], f32, isOutput=False)
    w2t = nc.declare_dram_parameter("w2t", [128, 8, 2], bf16, isOutput=False)
    b2t = nc.declare_dram_parameter("b2t", [2, 1], f32, isOutput=False)
    ident = nc.declare_dram_parameter("ident", [128, 128], f32, isOutput=False)
    out = nc.declare_dram_parameter("out", [2, 1], f32, isOutput=True)

    def dr_rhs(x_sb, a0, n):
        # overlapping DoubleRow rhs: slots read X[:, a0+j] and X[:, a0+j+1]
        v = x_sb[:, a0 : a0 + n].unsqueeze(1).broadcast_to([128, 2, n])
        v.ap = VecI64Pair([[SXP, 128], [1, 2], [1, n]])
        return v

    with tile.TileContext(nc) as tc:
        with (
            tc.tile_pool(name="const", bufs=1) as cpool,
            tc.tile_pool(name="gath", bufs=G) as gpool,
            tc.tile_pool(name="strip", bufs=4) as spool,
            tc.tile_pool(name="tree", bufs=2) as tpool,
            tc.tile_pool(name="dram", bufs=1, space="DRAM") as dpool,
        ):
            x_sb = cpool.tile([128, SXP], f8, name="x_sb")
            wdr_sb = cpool.tile([128, 2, 256], f8)
            wn_sb = cpool.tile([128, 2, 128], f8)
            widx_sb = cpool.tile([128, G], i32)
            wsT_sb = cpool.tile([128, 3, 4, 2 * D], bf16)
            ident_sb = cpool.tile([128, 128], f32)
            bsent_sb = cpool.tile([128, 4], f32)
            w1t_sb = cpool.tile([128, 4, 8, 128], bf16)
            b1t_sb = cpool.tile([128, 8], f32)
            w2t_sb = cpool.tile([128, 8, 2], bf16)
            b2t_sb = cpool.tile([2, 1], f32)
            u = cpool.tile([128, 4, UPAD], bf16, name="u")

            dum_in = dpool.tile([128, 2], f32)
            dum_out = nc.dram_tensor("dum_out", [CORES, 128, 2], f32, addr_space="Shared")

            # ---- head DMAs -------------------------------------------------
            # weights first (tiny) so bank-0 matmuls can start immediately
            nc.scalar.dma_start(out=wdr_sb[:], in_=wdr[:])
            nc.scalar.dma_start(out=wn_sb[:], in_=wn[:])
            # big sentence weights: one async transfer issued early
            nc.scalar.dma_start(out=wsT_sb[:], in_=wsT[:])
            # widx first so the word-emb gathers can start early
            nc.gpsimd.dma_start(out=widx_sb[:], in_=widx[:])
            # one-hot stream sections, alternating sync/gpsimd; first small
            secs = [0, 2 * 495 + 2]
            s = secs[-1]
            step = -(-(SXP - s) // 6)
            while s < SXP:
                s = min(s + step, SXP)
                secs.append(s)
            wrd_sb = [gpool.tile([128, D], f32, tag="wrd", name=f"wrd{g}") for g in range(G)]

            def gather(g):
                nc.gpsimd.indirect_dma_start(
                    out=wrd_sb[g][:],
                    out_offset=None,
                    in_=wemb[:],
                    in_offset=bass.IndirectOffsetOnAxis(ap=widx_sb[:, g : g + 1], axis=0),
                )

            gather(0)
            gather(1)
            for si, (a, b) in enumerate(zip(secs[:-1], secs[1:])):
                eng = nc.sync if si % 2 == 0 else nc.gpsimd
                eng.dma_start(out=x_sb[:, a:b], in_=onehot[:, a:b])
                if si == 1:
                    gather(2)
                if si == 3:
                    gather(3)
                    gather(4)
            nc.sync.dma_start(out=ident_sb[:], in_=ident[:])
            # warm-up collective: arms ncfw so the real AllGathers are cheap
            nc.gpsimd.dma_start(out=dum_in[:], in_=widx_sb[:].bitcast(f32)[:, 0:2])
            nc.gpsimd.collective_compute(
                "AllGather",
                mybir.AluOpType.bypass,
                replica_groups=[list(range(CORES))],
                ins=[dum_in[:]],
                outs=[dum_out[:]],
            )
            nc.gpsimd.dma_start(out=bsent_sb[:], in_=bsent[:])
            nc.gpsimd.dma_start(out=b1t_sb[:], in_=b1t[:])
            nc.gpsimd.dma_start(out=b2t_sb[:], in_=b2t[:])
            nc.gpsimd.dma_start(out=w1t_sb[:], in_=w1t[:])
            nc.gpsimd.dma_start(out=w2t_sb[:], in_=w2t[:])

            # ---- char pipeline + interleaved sentence conv -----------------
            rch = cpool.tile([128, 4, NCH], f32, name="rch")
            ccA_in = dpool.tile([128, 4], f32)
            ccB_in = dpool.tile([128, 4], f32)
            ccA_out = nc.dram_tensor("ccA_out", [CORES, 128, 4], f32, addr_space="Shared")
            ccB_out = nc.dram_tensor("ccB_out", [CORES, 128, 4], f32, addr_space="Shared")

            def char_bank(pch, m, w0, nw):
                n = PW * nw
                a0 = PW * w0
                py = pch.tile([128, 512], f32, tag="py")
                nc.tensor.matmul(
                    out=py[:, :n],
                    lhsT=wdr_sb[:, m, :],
                    rhs=dr_rhs(x_sb, a0, n),
                    start=True,
                    stop=False,
                    perf_mode=mybir.MatmulPerfMode.DoubleRowSwInterleave,
                )
                nc.tensor.matmul(
                    out=py[:, :n],
                    lhsT=wn_sb[:, m, :],
                    rhs=x_sb[:, a0 + 2 : a0 + 2 + n],
                    start=False,
                    stop=True,
                )
                return py

            def drain_direct(py, m, w0, nw):
                pv = py[:, : PW * nw].rearrange("p (w t) -> p w t", t=PW)
                nc.vector.tensor_reduce(
                    out=u[:, 2 + m, w0 : w0 + nw],
                    in_=pv[:, :, 0:L],
                    axis=mybir.AxisListType.X,
                    op=mybir.AluOpType.max,
                )

            def drain_evac(py, strip, j, nw):
                pv = py[:, : PW * nw].rearrange("p (w t) -> p w t", t=PW)
                nc.scalar.activation(
                    out=strip[:, j, :nw, :],
                    in_=pv[:, :, 0:L],
                    func=mybir.ActivationFunctionType.Copy,
                )

            def tree(strip, m, w0, nwords):
                # strip [128, 4, 15, 32] holding nwords valid words
                sv = strip[:].rearrange("p b w t -> p (b w) t")[:, :nwords, :]
                l1 = tpool.tile([128, 60, 16], bf16, tag="l1", name="l1")
                l2 = tpool.tile([128, 60, 8], bf16, tag="l2", name="l2")
                l3 = tpool.tile([128, 60, 4], bf16, tag="l3", name="l3")
                l4 = tpool.tile([128, 60, 2], bf16, tag="l4", name="l4")
                nc.vector.tensor_tensor(
                    out=l1[:, :nwords, :], in0=sv[:, :, 0:16], in1=sv[:, :, 16:32],
                    op=mybir.AluOpType.max)
                nc.vector.tensor_tensor(
                    out=l2[:, :nwords, :], in0=l1[:, :nwords, 0:8],
                    in1=l1[:, :nwords, 8:16], op=mybir.AluOpType.max)
                nc.vector.tensor_tensor(
                    out=l3[:, :nwords, :], in0=l2[:, :nwords, 0:4],
                    in1=l2[:, :nwords, 4:8], op=mybir.AluOpType.max)
                nc.vector.tensor_tensor(
                    out=l4[:, :nwords, :], in0=l3[:, :nwords, 0:2],
                    in1=l3[:, :nwords, 2:4], op=mybir.AluOpType.max)
                nc.vector.tensor_tensor(
                    out=u[:, 2 + m, w0 : w0 + nwords], in0=l4[:, :nwords, 0],
                    in1=l4[:, :nwords, 1], op=mybir.AluOpType.max)

            def transposes(ptp, g):
                wcnt = min(128, NW - g * 128)
                for cc in range(2):
                    tp = ptp.tile([128, 128], f32, tag="tp", name="tp")
                    nc.tensor.transpose(
                        out=tp[:],
                        in_=wrd_sb[g][:, cc * 128 : (cc + 1) * 128],
                        identity=ident_sb[:],
                    )
                    nc.scalar.activation(
                        out=u[:, cc, g * 128 : g * 128 + wcnt],
                        in_=tp[:, :wcnt],
                        func=mybir.ActivationFunctionType.Copy,
                    )

            def sent_mms(psn, c):
                # 48 matmul closures for sentence chunk c (woven between char
                # banks so their LDWEIGHTS hide under big char matmuls)
                n = min(CH, WPC - c * CH)
                # sentence PSUM reuses a char-bank-sized tile (512 f32 cols)
                ps_t = psn.tile([128, 4 * CH], f32, tag="py", name="ps")
                ps = ps_t[:].rearrange("p (m c) -> p m c", c=CH)
                items = []
                for mb in range(4):
                    for ki in range(12):
                        k, kc = divmod(ki, 4)
                        items.append((ps, mb, k, kc, n, ki == 0, ki == 11))
                return ps, n, items

            def emit_sent_mm(c, it):
                ps, mb, k, kc, n, first, last = it
                nc.tensor.matmul(
                    out=ps[:, mb, :n],
                    lhsT=wsT_sb[:, k, kc, mb * 128 : (mb + 1) * 128],
                    rhs=u[:, kc, c * CH + k : c * CH + k + n],
                    start=first,
                    stop=last,
                )

            def sent_reduce(ps, c, n):
                for mb in range(4):
                    nc.vector.tensor_reduce(
                        out=rch[:, mb, c : c + 1],
                        in_=ps[:, mb, :n],
                        axis=mybir.AxisListType.X,
                        op=mybir.AluOpType.max,
                    )

            def phase_a():
                # max over chunks 0..NCH-2 (+ sentence bias, folded here so
                # it needn't be added post-gather)
                ra = cpool.tile([128, 4], f32)
                nc.vector.tensor_reduce(
                    out=ra[:],
                    in_=rch[:, :, : NCH - 1],
                    axis=mybir.AxisListType.X,
                    op=mybir.AluOpType.max,
                )
                nc.vector.tensor_tensor(
                    out=ra[:], in0=ra[:], in1=bsent_sb[:], op=mybir.AluOpType.add
                )
                nc.gpsimd.dma_start(out=ccA_in[:], in_=ra[:])
                nc.gpsimd.collective_compute(
                    "AllGather",
                    mybir.AluOpType.bypass,
                    replica_groups=[list(range(CORES))],
                    ins=[ccA_in[:]],
                    outs=[ccA_out[:]],
                )

            # chunk c fires once u words [0, 128c+130) are done; with 75
            # words per supergroup that's after sg (128c+130)/75
            # transpose group g must precede the sentence chunk that reads
            # its u-word cols; schedule at sg 0/1/2
            tp_at_sg = {0: [0, 1], 1: [2, 3], 2: [4]} if G == 5 else {0: list(range(G))}
            # chunk c weaves into sg w(c): its u-deps (words < 128c+130) are
            # emitted AND have ~1 sg of pipeline slack by then (trees lag
            # their sg by one; sg w's bank-0 direct covers boundary words)
            weave_at_sg = {3: 0, 4: 1, 5: 2}

            rg = cpool.tile([128, 2, CORES, 4], f32, name="rg")

            with nc.named_scope("char"):
                with (
                    tc.tile_pool(name="pch", bufs=7, space="PSUM") as pch,
                    tc.tile_pool(name="ptp", bufs=1, space="PSUM") as ptp,
                ):
                    pending_trees = []
                    for sgi, sg in enumerate(sgs):
                        strips = []
                        for m in range(2):
                            if len(sg) > 1:
                                strips.append(spool.tile([128, 4, WB, L], bf16, tag=f"s{m}", name=f"strip{m}"))
                            else:
                                strips.append(None)
                        wv_c = weave_at_sg.get(sgi)
                        wv_items = []
                        if wv_c is not None:
                            wv_ps, wv_n, wv_items = sent_mms(pch, wv_c)
                        for bi, (w0, nw) in enumerate(sg):
                            for m in range(2):
                                py = char_bank(pch, m, w0, nw)
                                if bi == 0:
                                    drain_direct(py, m, w0, nw)
                                else:
                                    drain_evac(py, strips[m], bi - 1, nw)
                            if bi == 0:
                                # delayed trees: prior sg's strips are fully
                                # evacuated by now; emitting them here keeps
                                # the DVE queue from head-of-line blocking
                                for t in pending_trees:
                                    tree(*t)
                                pending_trees = []
                                for g in tp_at_sg.get(sgi, []):
                                    transposes(ptp, g)
                            elif wv_items:
                                # weave sentence matmuls between char banks
                                take = -(-len(wv_items) // ((5 - bi) * 2 - 1))
                                for _ in range(take):
                                    emit_sent_mm(wv_c, wv_items.pop(0))
                        if wv_c is not None:
                            for it in wv_items:
                                emit_sent_mm(wv_c, it)
                            sent_reduce(wv_ps, wv_c, wv_n)
                            if wv_c == NCH - 2:
                                phase_a()
                                nc.sync.dma_start(
                                    out=rg[:, 0], in_=ccA_out[:].rearrange("r p f -> p r f")
                                )
                        sg_w0 = sg[1][0] if len(sg) > 1 else None
                        sg_nw = sum(nw for (_, nw) in sg[1:])
                        for m in range(2):
                            if strips[m] is not None:
                                pending_trees.append((strips[m], m, sg_w0, sg_nw))
                    for t in pending_trees:
                        tree(*t)
                    # last sentence chunk (tail) + phase B (bias folded in)
                    ps3, n3, items3 = sent_mms(pch, NCH - 1)
                    for it in items3:
                        emit_sent_mm(NCH - 1, it)
                    sent_reduce(ps3, NCH - 1, n3)
                    rb = cpool.tile([128, 4], f32)
                    nc.vector.tensor_tensor(
                        out=rb[:],
                        in0=rch[:, :, NCH - 1 : NCH].rearrange("p m c -> p (m c)"),
                        in1=bsent_sb[:],
                        op=mybir.AluOpType.add,
                    )
                    nc.gpsimd.dma_start(out=ccB_in[:], in_=rb[:])
                    nc.gpsimd.collective_compute(
                        "AllGather",
                        mybir.AluOpType.bypass,
                        replica_groups=[list(range(CORES))],
                        ins=[ccB_in[:]],
                        outs=[ccB_out[:]],
                    )

            # ---- global max + MLP -----------------------------------------
            nc.sync.dma_start(out=rg[:, 1], in_=ccB_out[:].rearrange("r p f -> p r f"))
            r_sb = cpool.tile([128, 4], bf16)
            nc.vector.tensor_reduce(
                out=r_sb[:],
                in_=rg[:].rearrange("p g r f -> p f g r"),
                axis=mybir.AxisListType.XY,
                op=mybir.AluOpType.max,
            )
            with tc.tile_pool(name="pmlp", bufs=2, space="PSUM") as pmlp:
                hp = pmlp.tile([128, 8], f32, tag="hp")
                for j in range(8):
                    for k in range(4):
                        nc.tensor.matmul(
                            out=hp[:, j : j + 1],
                            lhsT=w1t_sb[:, k, j, :],
                            rhs=r_sb[:, k : k + 1],
                            start=(k == 0),
                            stop=(k == 3),
                        )
                hb = cpool.tile([128, 8], f32)
                nc.vector.tensor_tensor(
                    out=hb[:], in0=hp[:], in1=b1t_sb[:], op=mybir.AluOpType.add
                )
                h_sb = cpool.tile([128, 8], bf16)
                nc.scalar.activation(
                    out=h_sb[:], in_=hb[:], func=mybir.ActivationFunctionType.Tanh
                )
                o_ps = pmlp.tile([2, 1], f32, tag="op")
                for k in range(8):
                    nc.tensor.matmul(
                        out=o_ps[:],
                        lhsT=w2t_sb[:, k, :],
                        rhs=h_sb[:, k : k + 1],
                        start=(k == 0),
                        stop=(k == 7),
                    )
                o_sb = cpool.tile([2, 1], f32)
                nc.vector.tensor_tensor(
                    out=o_sb[:], in0=o_ps[:], in1=b2t_sb[:], op=mybir.AluOpType.add
                )
                nc.sync.dma_start(out=out[:], in_=o_sb[:])

    nc.finalize()
    return nc


def prep_in_maps(words, words_in_char, word_emb, chr_emb, conv_chr_w, conv_chr_b,
                 conv_sent_w, conv_sent_b, w1, b1, w2, b2):
    W = words.shape[0]
    WPC, NW, SX, SXP, NB, G = _shapes(W)

    words = np.asarray(words, np.int32)
    chars = np.asarray(words_in_char, np.int32)
    word_emb = np.asarray(word_emb, np.float32)
    chr_emb = np.asarray(chr_emb, np.float32)
    conv_chr_w = np.asarray(conv_chr_w, np.float32)
    conv_chr_b = np.asarray(conv_chr_b, np.float32)
    conv_sent_w = np.asarray(conv_sent_w, np.float32)
    conv_sent_b = np.asarray(conv_sent_b, np.float32)
    w1 = np.asarray(w1, np.float32)
    b1 = np.asarray(b1, np.float32)
    w2 = np.asarray(w2, np.float32)
    b2 = np.asarray(b2, np.float32)

    # host-folded char response tables (x64 into fp8 range); conv bias is
    # folded into the center tap's table (center col is never a pad col)
    ET = [ETS * (chr_emb @ conv_chr_w[:, :, k].T) for k in range(3)]
    ET[1] = ET[1] + ETS * conv_chr_b[None, :]
    ET = [t.astype(FP8).astype(np.float32) for t in ET]

    # DR SWI weights for (ET0, ET1'), per m-half; normal weights = ET2
    wdr = np.zeros((128, 2, 256), np.float32)
    wn = np.zeros((128, 2, 128), np.float32)
    for m in range(2):
        a = ET[0][:, m * 128 : (m + 1) * 128]
        b = ET[1][:, m * 128 : (m + 1) * 128]
        wdr[:, m, :] = np.stack([a[:, ::-1], b[:, ::-1]], axis=2).reshape(128, 256)
        wn[:, m, :] = ET[2][:, m * 128 : (m + 1) * 128]

    # sentence conv: char-half input channels absorb the 1/64
    ws = conv_sent_w.copy()
    ws[:, D:, :] /= ETS
    wsT = np.ascontiguousarray(
        ws.transpose(1, 2, 0).reshape(4, 128, 3, 2 * D).transpose(1, 2, 0, 3)
    ).astype(BF16)                                        # [p, k, kc, c2]
    bsent = np.ascontiguousarray(conv_sent_b.reshape(4, 128).T).astype(np.float32)
    w1t = np.ascontiguousarray(
        w1.reshape(8, 128, 4, 128).transpose(3, 2, 0, 1)
    ).astype(BF16)                                  # [p, k, m, c]
    b1t = np.ascontiguousarray(b1.reshape(8, 128).T).astype(np.float32)
    w2t = np.ascontiguousarray(
        w2.T.reshape(8, 128, 2).transpose(1, 0, 2)
    ).astype(BF16)                                  # [p, k, j]
    b2t = b2.reshape(2, 1).astype(np.float32)
    ident = np.eye(128, dtype=np.float32)

    in_maps = []
    for c in range(CORES):
        lo = c * WPC - 1
        idxs = np.arange(lo, lo + NW)
        valid = (idxs >= 0) & (idxs < W)
        w_ext = np.where(valid, words[np.clip(idxs, 0, W - 1)], 0).astype(np.int32)
        ch_ext = np.zeros((NW, L), np.int32)
        ch_ext[valid] = chars[np.clip(idxs, 0, W - 1)[valid]]

        # one-hot stream: word w chars at cols PW*w+1 .. PW*w+32; halo words
        # outside [0, W) stay all-zero (their u must be exactly 0)
        oh = np.zeros((128, SXP), FP8)
        cols = (PW * np.arange(NW)[:, None] + 1 + np.arange(L)[None, :])
        vmask = np.repeat(valid, L)
        cf = cols.reshape(-1)[vmask]
        chf = ch_ext.reshape(-1)[vmask]
        oh[chf, cf] = 1.0

        wpad = np.zeros(G * 128, np.int32)
        wpad[:NW] = w_ext
        widx = np.ascontiguousarray(wpad.reshape(G, 128).T)

        in_maps.append(
            dict(
                onehot=oh,
                wdr=wdr.astype(FP8),
                wn=wn.astype(FP8),
                widx=widx,
                wemb=word_emb,
                wsT=wsT,
                bsent=bsent,
                w1t=w1t,
                b1t=b1t,
                w2t=w2t,
                b2t=b2t,
                ident=ident,
            )
        )
    return in_maps


_CACHE = {}


def _get_nc(W):
    if W not in _CACHE:
        _CACHE[W] = build(W)
    return _CACHE[W]


def run(inputs, trace=False):
    W = np.asarray(inputs["words"]).shape[0]
    nc = _get_nc(W)
    in_maps = prep_in_maps(**inputs)
    res = run_bass_kernel_spmd(nc, in_maps, list(range(CORES)), trace=trace)
    out = np.asarray(res.results[0]["out"], np.float32).reshape(1, 2)
    return out, res


def kernel(**inputs) -> np.ndarray:
    out, _ = run(inputs, trace=False)
    return out


# revision 18
# speedup vs baseline: 1.2645x; 1.0111x over previous
"""Trainium2 Bass kernel for nn_ConvNet (char-CNN word encoder + sentence conv + MLP).

Single-stream char path: one one-hot stream X[128 vocab, SX] fp8 with one
zero pad column between words (33 cols/word).  The 3-tap conv collapses to
shifted one-hot matmuls into the same PSUM bank:
    out[j] = ET0.X[j] + ET1'.X[j+1] + ET2.X[j+2]
where ET1' has the conv bias folded in (center tap is never a pad col for a
real output).  The DoubleRow fp8 pass computes (ET0, ET1') via an
OVERLAPPING rhs AP (slots read X[j], X[j+1] from the same stream); the
third tap is a normal fp8 matmul on X[j+2:].  Word isolation is automatic:
pad cols contribute exactly 0, and outputs at pad positions (j%33==32) are
garbage that the strided max simply skips.

PSUM drain (the DVE bottleneck in v1) is split: per 5-bank supergroup, one
bank is max-reduced directly from PSUM by the DVE; four banks are evacuated
by the Scalar engine (strided compact copy to bf16 SBUF) and reduced by a
DVE tensor_tensor max tree, which runs in the 2x_1p DVE mode (2 elem/cyc)
unlike tensor_reduce (1x only).

The sentence conv (bf16, 48 matmuls) is interleaved into the char phase in
4 chunks of 128 words so the PE never idles (p-state stays at 2.4 GHz) and
only the last chunk is tail-exposed.  The 8-way max uses two AllGathers of
[128,4] partial maxes: phase A (chunks 0-2) launches ~85% through the char
phase and hides its latency; phase B (chunk 3) pays only its own sync.
"""

import sys

try:
    import concourse  # noqa: F401
except ImportError:
    sys.path.insert(0, "/opt/trn_rl_repo")

import numpy as np
import ml_dtypes

import concourse.bass as bass
import concourse.bacc as bacc
import concourse.tile as tile
from concourse import mybir
from concourse.bass_utils import run_bass_kernel_spmd
from bass_rust import VecI64Pair

BF16 = ml_dtypes.bfloat16
FP8 = ml_dtypes.float8_e4m3

CORES = 8
D = 256
L = 32
PW = L + 1          # stream cols per word (32 chars + 1 pad)
WB = 15             # words per PSUM bank (15*33 = 495 <= 512)
ETS = 64.0          # fp8 scale for the ET response tables


def _shapes(W):
    WPC = W // CORES            # real words per core
    NW = WPC + 2                # + 1 halo word each side
    SX = PW * NW + 1 + 2        # stream cols (+lead pad, +2 tail zeros)
    SXP = -(-SX // 16) * 16     # padded to 16
    NB = -(-NW // WB)           # banks per m-half
    G = -(-NW // 128)           # word-gather groups of 128
    return WPC, NW, SX, SXP, NB, G


def build(W):
    WPC, NW, SX, SXP, NB, G = _shapes(W)
    f32 = mybir.dt.float32
    bf16 = mybir.dt.bfloat16
    f8 = mybir.dt.float8e4
    i32 = mybir.dt.int32

    # banks: (word0, nwords); supergroups of 5 banks (last ragged)
    banks = []
    w0 = 0
    while w0 < NW:
        banks.append((w0, min(WB, NW - w0)))
        w0 += WB
    sgs = [banks[i : i + 5] for i in range(0, len(banks), 5)]

    # sentence chunks of ~WPC/3 real words; chunk c ready after u words
    # [0, CH(c+1)+2) exist
    CH = 128
    NCH = -(-WPC // CH)
    UPAD = -(-NW // 16) * 16

    nc = bacc.Bacc(num_devices=CORES)

    onehot = nc.declare_dram_parameter("onehot", [128, SXP], f8, isOutput=False)
    wdr = nc.declare_dram_parameter("wdr", [128, 2, 256], f8, isOutput=False)
    wn = nc.declare_dram_parameter("wn", [128, 2, 128], f8, isOutput=False)
    widx = nc.declare_dram_parameter("widx", [128, G], i32, isOutput=False)
    wemb = nc.declare_dram_parameter("wemb", [50000, D], f32, isOutput=False)
    wsT = nc.declare_dram_parameter("wsT", [128, 3, 4, 2 * D], bf16, isOutput=False)
    bsent = nc.declare_dram_parameter("bsent", [128, 4], f32, isOutput=False)
    w1t = nc.declare_dram_parameter("w1t", [128, 4, 8, 128], bf16, isOutput=False)
    b1t = nc.declare_dram_parameter("b1t", [128, 8], f32, isOutput=False)
    w2t = nc.declare_dram_parameter("w2t", [128, 8, 2], bf16, isOutput=False)
    b2t = nc.declare_dram_parameter("b2t", [2, 1], f32, isOutput=False)
    ident = nc.declare_dram_parameter("ident", [128, 128], f32, isOutput=False)
    out = nc.declare_dram_parameter("out", [2, 1], f32, isOutput=True)

    def dr_rhs(x_sb, a0, n):
        # overlapping DoubleRow rhs: slots read X[:, a0+j] and X[:, a0+j+1]
        v = x_sb[:, a0 : a0 + n].unsqueeze(1).broadcast_to([128, 2, n])
        v.ap = VecI64Pair([[SXP, 128], [1, 2], [1, n]])
        return v

    with tile.TileContext(nc) as tc:
        with (
            tc.tile_pool(name="const", bufs=1) as cpool,
            tc.tile_pool(name="gath", bufs=G) as gpool,
            tc.tile_pool(name="strip", bufs=4) as spool,
            tc.tile_pool(name="tree", bufs=2) as tpool,
            tc.tile_pool(name="dram", bufs=1, space="DRAM") as dpool,
        ):
            x_sb = cpool.tile([128, SXP], f8, name="x_sb")
            wdr_sb = cpool.tile([128, 2, 256], f8)
            wn_sb = cpool.tile([128, 2, 128], f8)
            widx_sb = cpool.tile([128, G], i32)
            wsT_sb = cpool.tile([128, 3, 4, 2 * D], bf16)
            ident_sb = cpool.tile([128, 128], f32)
            bsent_sb = cpool.tile([128, 4], f32)
            w1t_sb = cpool.tile([128, 4, 8, 128], bf16)
            b1t_sb = cpool.tile([128, 8], f32)
            w2t_sb = cpool.tile([128, 8, 2], bf16)
            b2t_sb = cpool.tile([2, 1], f32)
            u = cpool.tile([128, 4, UPAD], bf16, name="u")

            dum_in = dpool.tile([128, 2], f32)
            dum_out = nc.dram_tensor("dum_out", [CORES, 128, 2], f32, addr_space="Shared")

            # ---- head DMAs -------------------------------------------------
            # weights first (tiny) so bank-0 matmuls can start immediately
            nc.scalar.dma_start(out=wdr_sb[:], in_=wdr[:])
            nc.scalar.dma_start(out=wn_sb[:], in_=wn[:])
            # big sentence weights: one async transfer issued early
            nc.scalar.dma_start(out=wsT_sb[:], in_=wsT[:])
            # widx first so the word-emb gathers can start early
            nc.gpsimd.dma_start(out=widx_sb[:], in_=widx[:])
            # one-hot stream sections, alternating sync/gpsimd; first small
            secs = [0, 2 * 495 + 2]
            s = secs[-1]
            step = -(-(SXP - s) // 6)
            while s < SXP:
                s = min(s + step, SXP)
                secs.append(s)
            wrd_sb = [gpool.tile([128, D], f32, tag="wrd", name=f"wrd{g}") for g in range(G)]

            def gather(g):
                nc.gpsimd.indirect_dma_start(
                    out=wrd_sb[g][:],
                    out_offset=None,
                    in_=wemb[:],
                    in_offset=bass.IndirectOffsetOnAxis(ap=widx_sb[:, g : g + 1], axis=0),
                )

            gather(0)
            gather(1)
            for si, (a, b) in enumerate(zip(secs[:-1], secs[1:])):
                eng = nc.sync if si % 2 == 0 else nc.gpsimd
                eng.dma_start(out=x_sb[:, a:b], in_=onehot[:, a:b])
                if si == 1:
                    gather(2)
                if si == 3:
                    gather(3)
                    gather(4)
            nc.sync.dma_start(out=ident_sb[:], in_=ident[:])
            # warm-up collective: arms ncfw so the real AllGathers are cheap
            nc.gpsimd.dma_start(out=dum_in[:], in_=widx_sb[:].bitcast(f32)[:, 0:2])
            nc.gpsimd.collective_compute(
                "AllGather",
                mybir.AluOpType.bypass,
                replica_groups=[list(range(CORES))],
                ins=[dum_in[:]],
                outs=[dum_out[:]],
            )
            nc.gpsimd.dma_start(out=bsent_sb[:], in_=bsent[:])
            nc.gpsimd.dma_start(out=b1t_sb[:], in_=b1t[:])
            nc.gpsimd.dma_start(out=b2t_sb[:], in_=b2t[:])
            nc.gpsimd.dma_start(out=w1t_sb[:], in_=w1t[:])
            nc.gpsimd.dma_start(out=w2t_sb[:], in_=w2t[:])

            # ---- char pipeline + interleaved sentence conv -----------------
            rch = cpool.tile([128, 4, NCH], f32, name="rch")
            ccA_in = dpool.tile([128, 4], f32)
            ccB_in = dpool.tile([128, 4], f32)

            def char_bank(pch, m, w0, nw):
                n = PW * nw
                a0 = PW * w0
                py = pch.tile([128, 512], f32, tag="py")
                nc.tensor.matmul(
                    out=py[:, :n],
                    lhsT=wdr_sb[:, m, :],
                    rhs=dr_rhs(x_sb, a0, n),
                    start=True,
                    stop=False,
                    perf_mode=mybir.MatmulPerfMode.DoubleRowSwInterleave,
                )
                nc.tensor.matmul(
                    out=py[:, :n],
                    lhsT=wn_sb[:, m, :],
                    rhs=x_sb[:, a0 + 2 : a0 + 2 + n],
                    start=False,
                    stop=True,
                )
                return py

            def drain_direct(py, m, w0, nw):
                pv = py[:, : PW * nw].rearrange("p (w t) -> p w t", t=PW)
                nc.vector.tensor_reduce(
                    out=u[:, 2 + m, w0 : w0 + nw],
                    in_=pv[:, :, 0:L],
                    axis=mybir.AxisListType.X,
                    op=mybir.AluOpType.max,
                )

            def drain_evac(py, strip, j, nw):
                pv = py[:, : PW * nw].rearrange("p (w t) -> p w t", t=PW)
                nc.scalar.activation(
                    out=strip[:, j, :nw, :],
                    in_=pv[:, :, 0:L],
                    func=mybir.ActivationFunctionType.Copy,
                )

            def tree(strip, m, w0, nwords):
                # strip [128, 4, 15, 32] holding nwords valid words
                sv = strip[:].rearrange("p b w t -> p (b w) t")[:, :nwords, :]
                l1 = tpool.tile([128, 60, 16], bf16, tag="l1", name="l1")
                l2 = tpool.tile([128, 60, 8], bf16, tag="l2", name="l2")
                l3 = tpool.tile([128, 60, 4], bf16, tag="l3", name="l3")
                l4 = tpool.tile([128, 60, 2], bf16, tag="l4", name="l4")
                nc.vector.tensor_tensor(
                    out=l1[:, :nwords, :], in0=sv[:, :, 0:16], in1=sv[:, :, 16:32],
                    op=mybir.AluOpType.max)
                nc.vector.tensor_tensor(
                    out=l2[:, :nwords, :], in0=l1[:, :nwords, 0:8],
                    in1=l1[:, :nwords, 8:16], op=mybir.AluOpType.max)
                nc.vector.tensor_tensor(
                    out=l3[:, :nwords, :], in0=l2[:, :nwords, 0:4],
                    in1=l2[:, :nwords, 4:8], op=mybir.AluOpType.max)
                nc.vector.tensor_tensor(
                    out=l4[:, :nwords, :], in0=l3[:, :nwords, 0:2],
                    in1=l3[:, :nwords, 2:4], op=mybir.AluOpType.max)
                nc.vector.tensor_tensor(
                    out=u[:, 2 + m, w0 : w0 + nwords], in0=l4[:, :nwords, 0],
                    in1=l4[:, :nwords, 1], op=mybir.AluOpType.max)

            def transposes(ptp, g):
                wcnt = min(128, NW - g * 128)
                for cc in range(2):
                    tp = ptp.tile([128, 128], f32, tag="tp", name="tp")
                    nc.tensor.transpose(
                        out=tp[:],
                        in_=wrd_sb[g][:, cc * 128 : (cc + 1) * 128],
                        identity=ident_sb[:],
                    )
                    nc.scalar.activation(
                        out=u[:, cc, g * 128 : g * 128 + wcnt],
                        in_=tp[:, :wcnt],
                        func=mybir.ActivationFunctionType.Copy,
                    )

            def sent_mms(psn, c):
                # 48 matmul closures for sentence chunk c (woven between char
                # banks so their LDWEIGHTS hide under big char matmuls)
                n = min(CH, WPC - c * CH)
                # sentence PSUM reuses a char-bank-sized tile (512 f32 cols)
                ps_t = psn.tile([128, 4 * CH], f32, tag="py", name="ps")
                ps = ps_t[:].rearrange("p (m c) -> p m c", c=CH)
                items = []
                for mb in range(4):
                    for ki in range(12):
                        k, kc = divmod(ki, 4)
                        items.append((ps, mb, k, kc, n, ki == 0, ki == 11))
                return ps, n, items

            def emit_sent_mm(c, it):
                ps, mb, k, kc, n, first, last = it
                nc.tensor.matmul(
                    out=ps[:, mb, :n],
                    lhsT=wsT_sb[:, k, kc, mb * 128 : (mb + 1) * 128],
                    rhs=u[:, kc, c * CH + k : c * CH + k + n],
                    start=first,
                    stop=last,
                )

            def sent_reduce(ps, c, n):
                for mb in range(4):
                    nc.vector.tensor_reduce(
                        out=rch[:, mb, c : c + 1],
                        in_=ps[:, mb, :n],
                        axis=mybir.AxisListType.X,
                        op=mybir.AluOpType.max,
                    )

            def phase_a():
                # max over chunks 0..NCH-2 (+ sentence bias, folded here so
                # it needn't be added post-gather)
                ra = cpool.tile([128, 4], f32)
                nc.vector.tensor_reduce(
                    out=ra[:],
                    in_=rch[:, :, : NCH - 1],
                    axis=mybir.AxisListType.X,
                    op=mybir.AluOpType.max,
                )
                nc.vector.tensor_tensor(
                    out=ra[:], in0=ra[:], in1=bsent_sb[:], op=mybir.AluOpType.add
                )
                nc.gpsimd.dma_start(out=ccA_in[:], in_=ra[:])
                nc.gpsimd.collective_compute(
                    "AllReduce",
                    mybir.AluOpType.max,
                    replica_groups=[list(range(CORES))],
                    ins=[ccA_in[:]],
                    outs=[ccA_in[:]],
                )

            # chunk c fires once u words [0, 128c+130) are done; with 75
            # words per supergroup that's after sg (128c+130)/75
            # transpose group g must precede the sentence chunk that reads
            # its u-word cols; schedule at sg 0/1/2
            tp_at_sg = {0: [0, 1], 1: [2, 3], 2: [4]} if G == 5 else {0: list(range(G))}
            # chunk c weaves into sg w(c): its u-deps (words < 128c+130) are
            # emitted AND have ~1 sg of pipeline slack by then (trees lag
            # their sg by one; sg w's bank-0 direct covers boundary words)
            weave_at_sg = {3: 0, 4: 1, 5: 2}

            rgA = cpool.tile([128, 4], f32, name="rgA")
            rgB = cpool.tile([128, 4], f32, name="rgB")

            with nc.named_scope("char"):
                with (
                    tc.tile_pool(name="pch", bufs=7, space="PSUM") as pch,
                    tc.tile_pool(name="ptp", bufs=1, space="PSUM") as ptp,
                ):
                    pending_trees = []
                    for sgi, sg in enumerate(sgs):
                        strips = []
                        for m in range(2):
                            if len(sg) > 1:
                                strips.append(spool.tile([128, 4, WB, L], bf16, tag=f"s{m}", name=f"strip{m}"))
                            else:
                                strips.append(None)
                        wv_c = weave_at_sg.get(sgi)
                        wv_items = []
                        if wv_c is not None:
                            wv_ps, wv_n, wv_items = sent_mms(pch, wv_c)
                        for bi, (w0, nw) in enumerate(sg):
                            for m in range(2):
                                py = char_bank(pch, m, w0, nw)
                                if bi == 0:
                                    drain_direct(py, m, w0, nw)
                                else:
                                    drain_evac(py, strips[m], bi - 1, nw)
                            if bi == 0:
                                # delayed trees: prior sg's strips are fully
                                # evacuated by now; emitting them here keeps
                                # the DVE queue from head-of-line blocking
                                for t in pending_trees:
                                    tree(*t)
                                pending_trees = []
                                for g in tp_at_sg.get(sgi, []):
                                    transposes(ptp, g)
                            elif wv_items:
                                # weave sentence matmuls between char banks
                                take = -(-len(wv_items) // ((5 - bi) * 2 - 1))
                                for _ in range(take):
                                    emit_sent_mm(wv_c, wv_items.pop(0))
                        if wv_c is not None:
                            for it in wv_items:
                                emit_sent_mm(wv_c, it)
                            sent_reduce(wv_ps, wv_c, wv_n)
                            if wv_c == NCH - 2:
                                phase_a()
                                nc.sync.dma_start(out=rgA[:], in_=ccA_in[:])
                        sg_w0 = sg[1][0] if len(sg) > 1 else None
                        sg_nw = sum(nw for (_, nw) in sg[1:])
                        for m in range(2):
                            if strips[m] is not None:
                                pending_trees.append((strips[m], m, sg_w0, sg_nw))
                    for t in pending_trees:
                        tree(*t)
                    # last sentence chunk (tail) + phase B (bias folded in)
                    ps3, n3, items3 = sent_mms(pch, NCH - 1)
                    for it in items3:
                        emit_sent_mm(NCH - 1, it)
                    sent_reduce(ps3, NCH - 1, n3)
                    rb = cpool.tile([128, 4], f32)
                    nc.vector.tensor_tensor(
                        out=rb[:],
                        in0=rch[:, :, NCH - 1 : NCH].rearrange("p m c -> p (m c)"),
                        in1=bsent_sb[:],
                        op=mybir.AluOpType.add,
                    )
                    nc.gpsimd.dma_start(out=ccB_in[:], in_=rb[:])
                    nc.gpsimd.collective_compute(
                        "AllReduce",
                        mybir.AluOpType.max,
                        replica_groups=[list(range(CORES))],
                        ins=[ccB_in[:]],
                        outs=[ccB_in[:]],
                    )

            # ---- global max + MLP -----------------------------------------
            nc.sync.dma_start(out=rgB[:], in_=ccB_in[:])
            r_sb = cpool.tile([128, 4], bf16)
            nc.vector.tensor_tensor(
                out=r_sb[:], in0=rgA[:], in1=rgB[:], op=mybir.AluOpType.max
            )
            with tc.tile_pool(name="pmlp", bufs=2, space="PSUM") as pmlp:
                hp = pmlp.tile([128, 8], f32, tag="hp")
                for j in range(8):
                    for k in range(4):
                        nc.tensor.matmul(
                            out=hp[:, j : j + 1],
                            lhsT=w1t_sb[:, k, j, :],
                            rhs=r_sb[:, k : k + 1],
                            start=(k == 0),
                            stop=(k == 3),
                        )
                hb = cpool.tile([128, 8], f32)
                nc.vector.tensor_tensor(
                    out=hb[:], in0=hp[:], in1=b1t_sb[:], op=mybir.AluOpType.add
                )
                h_sb = cpool.tile([128, 8], bf16)
                nc.scalar.activation(
                    out=h_sb[:], in_=hb[:], func=mybir.ActivationFunctionType.Tanh
                )
                o_ps = pmlp.tile([2, 1], f32, tag="op")
                for k in range(8):
                    nc.tensor.matmul(
                        out=o_ps[:],
                        lhsT=w2t_sb[:, k, :],
                        rhs=h_sb[:, k : k + 1],
                        start=(k == 0),
                        stop=(k == 7),
                    )
                o_sb = cpool.tile([2, 1], f32)
                nc.vector.tensor_tensor(
                    out=o_sb[:], in0=o_ps[:], in1=b2t_sb[:], op=mybir.AluOpType.add
                )
                nc.sync.dma_start(out=out[:], in_=o_sb[:])

    nc.finalize()
    return nc


def prep_in_maps(words, words_in_char, word_emb, chr_emb, conv_chr_w, conv_chr_b,
                 conv_sent_w, conv_sent_b, w1, b1, w2, b2):
    W = words.shape[0]
    WPC, NW, SX, SXP, NB, G = _shapes(W)

    words = np.asarray(words, np.int32)
    chars = np.asarray(words_in_char, np.int32)
    word_emb = np.asarray(word_emb, np.float32)
    chr_emb = np.asarray(chr_emb, np.float32)
    conv_chr_w = np.asarray(conv_chr_w, np.float32)
    conv_chr_b = np.asarray(conv_chr_b, np.float32)
    conv_sent_w = np.asarray(conv_sent_w, np.float32)
    conv_sent_b = np.asarray(conv_sent_b, np.float32)
    w1 = np.asarray(w1, np.float32)
    b1 = np.asarray(b1, np.float32)
    w2 = np.asarray(w2, np.float32)
    b2 = np.asarray(b2, np.float32)

    # host-folded char response tables (x64 into fp8 range); conv bias is
    # folded into the center tap's table (center col is never a pad col)
    ET = [ETS * (chr_emb @ conv_chr_w[:, :, k].T) for k in range(3)]
    ET[1] = ET[1] + ETS * conv_chr_b[None, :]
    ET = [t.astype(FP8).astype(np.float32) for t in ET]

    # DR SWI weights for (ET0, ET1'), per m-half; normal weights = ET2
    wdr = np.zeros((128, 2, 256), np.float32)
    wn = np.zeros((128, 2, 128), np.float32)
    for m in range(2):
        a = ET[0][:, m * 128 : (m + 1) * 128]
        b = ET[1][:, m * 128 : (m + 1) * 128]
        wdr[:, m, :] = np.stack([a[:, ::-1], b[:, ::-1]], axis=2).reshape(128, 256)
        wn[:, m, :] = ET[2][:, m * 128 : (m + 1) * 128]

    # sentence conv: char-half input channels absorb the 1/64
    ws = conv_sent_w.copy()
    ws[:, D:, :] /= ETS
    wsT = np.ascontiguousarray(
        ws.transpose(1, 2, 0).reshape(4, 128, 3, 2 * D).transpose(1, 2, 0, 3)
    ).astype(BF16)                                        # [p, k, kc, c2]
    bsent = np.ascontiguousarray(conv_sent_b.reshape(4, 128).T).astype(np.float32)
    w1t = np.ascontiguousarray(
        w1.reshape(8, 128, 4, 128).transpose(3, 2, 0, 1)
    ).astype(BF16)                                  # [p, k, m, c]
    b1t = np.ascontiguousarray(b1.reshape(8, 128).T).astype(np.float32)
    w2t = np.ascontiguousarray(
        w2.T.reshape(8, 128, 2).transpose(1, 0, 2)
    ).astype(BF16)                                  # [p, k, j]
    b2t = b2.reshape(2, 1).astype(np.float32)
    ident = np.eye(128, dtype=np.float32)

    in_maps = []
    for c in range(CORES):
        lo = c * WPC - 1
        idxs = np.arange(lo, lo + NW)
        valid = (idxs >= 0) & (idxs < W)
        w_ext = np.where(valid, words[np.clip(idxs, 0, W - 1)], 0).astype(np.int32)
        ch_ext = np.zeros((NW, L), np.int32)
        ch_ext[valid] = chars[np.clip(idxs, 0, W - 1)[valid]]

        # one-hot stream: word w chars at cols PW*w+1 .. PW*w+32; halo words
        # outside [0, W) stay all-zero (their u must be exactly 0)
        oh = np.zeros((128, SXP), FP8)
        cols = (PW * np.arange(NW)[:, None] + 1 + np.arange(L)[None, :])
        vmask = np.repeat(valid, L)
        cf = cols.reshape(-1)[vmask]
        chf = ch_ext.reshape(-1)[vmask]
        oh[chf, cf] = 1.0

        wpad = np.zeros(G * 128, np.int32)
        wpad[:NW] = w_ext
        widx = np.ascontiguousarray(wpad.reshape(G, 128).T)

        in_maps.append(
            dict(
                onehot=oh,
                wdr=wdr.astype(FP8),
                wn=wn.astype(FP8),
                widx=widx,
                wemb=word_emb,
                wsT=wsT,
                bsent=bsent,
                w1t=w1t,
                b1t=b1t,
                w2t=w2t,
                b2t=b2t,
                ident=ident,
            )
        )
    return in_maps


_CACHE = {}


def _get_nc(W):
    if W not in _CACHE:
        _CACHE[W] = build(W)
    return _CACHE[W]


def run(inputs, trace=False):
    W = np.asarray(inputs["words"]).shape[0]
    nc = _get_nc(W)
    in_maps = prep_in_maps(**inputs)
    res = run_bass_kernel_spmd(nc, in_maps, list(range(CORES)), trace=trace)
    out = np.asarray(res.results[0]["out"], np.float32).reshape(1, 2)
    return out, res


def kernel(**inputs) -> np.ndarray:
    out, _ = run(inputs, trace=False)
    return out
